# revision 1
# baseline (speedup 1.0000x reference)
"""TRN2 Bass/Tile kernel for nn_DHSMBlock (SSM + self-attn + hierarchical memory + FFN).

Sharding: data-parallel over batch. B=8 rows -> 8 NeuronCores, one row per core,
no collectives. Each core gets the full weight set (host pre-transposed).

On-device layout is feature-major: every activation lives as X^T [feature, token]
so that all matmuls contract over the partition dim. Weights are shipped as W^T
[in_f, out_f] (host numpy transpose). LayerNorm is over the feature dim =
partition dim; stats are computed with ones-vector matmuls on the PE and
broadcast back with SBUF->SBUF stride-0 DMAs. The SSM recurrence is a single
DVE tensor_tensor_scan instruction per core. Softmax is computed k-major
(scores^T), so no transposes are needed anywhere except the final output.
"""

import os
from contextlib import ExitStack

import numpy as np

os.environ.setdefault("MYCRO_LOCAL_CACHE", "1")

import concourse.bass as bass
import concourse.mybir as mybir
import concourse.tile as tile
from concourse import bass_utils
from concourse.masks import make_identity

F32 = mybir.dt.float32
FR = mybir.dt.float32r
BF16 = mybir.dt.bfloat16
AF = mybir.ActivationFunctionType
OP = mybir.AluOpType

B, T, H, S = 8, 1024, 1024, 128
NH, DH = 16, 64          # self-attention heads
RH, RDH = 4, 256         # retriever heads
COMP = [1024, 512, 256]  # compressor widths
P = 128
HT = H // P              # 8 feature tiles
NCH = 512                # matmul moving-dim chunk (one fp32 PSUM bank)
EPS = 1e-5


def _fr(ap):
    return ap.bitcast(FR)


def build_nc(fl):
    """Build the Bass program. fl: dict of host-known triviality flags."""
    nc = bass.Bass("TRN2", target_bir_lowering=False, debug=False, num_devices=8)
    D = {}

    def din(name, shape, dt=F32):
        D[name] = nc.dram_tensor(name, list(shape), dt, kind="ExternalInput").ap()

    din("xT", (H, T), FR)
    for i in range(3):
        din(f"m{i}T", (H, 256), FR)
    din("wsgT", (H, S), FR); din("wBT", (H, S), FR); din("wCT", (S, H), FR)
    din("A", (S, 1)); din("sg_b", (S, 1)); din("Dp1", (H, 1))
    din("outp_wT", (H, H), FR); din("outp_b", (H, 1))
    din("a_wqT", (H, H), FR); din("a_wkT", (H, H), FR); din("a_wvT", (H, H), FR)
    din("a_bq", (H, 1)); din("a_woT", (H, H), FR); din("a_const", (H, 1))
    for i, c in enumerate(COMP):
        din(f"c{i}_w1T", (H, c), FR); din(f"c{i}_b1", (c, 1))
        din(f"c{i}_w2T", (c, H), FR); din(f"c{i}_b2", (H, 1))
    din("r_wqT", (H, H), FR); din("r_wkT", (H, H), FR); din("r_wvT", (H, H), FR)
    din("r_bq", (H, 1)); din("r_woT", (H, H), FR); din("r_const", (H, 1))
    din("mg_wT", (2 * H, H), FR); din("mg_b", (H, 1))
    din("f_w1T", (H, 4 * H), FR); din("f_b1", (4 * H, 1))
    din("f_w2T", (4 * H, H), BF16); din("f_b2", (H, 1))
    for n in ("sln", "n1", "n2", "n3"):
        din(f"{n}_g", (H, 1)); din(f"{n}_b", (H, 1))
    din("ones", (P, 1), FR)
    din("ones16", (P, NH), FR)
    out_d = nc.dram_tensor("out", [T, H], F32, kind="ExternalOutput").ap()

    with tile.TileContext(nc, pool_alloc_mode="queue") as tc:
        _body(nc, tc, D, out_d, fl)
    _split_matmul_waits(nc)
    return nc


_WAIT_EXEMPT = {
    "InstEventSemaphore", "InstAllEngineBarrier",
    "InstUnconditionalBranch", "InstCompareAndBranch", "InstIndirectBranch",
    "InstHalt", "InstBranchHint",
}


def _split_matmul_waits(nc):
    """TPB engine instruction encodings carry at most one sync wait; move
    surplus waits onto a preceding same-engine no-op (sequencer WAITs)."""
    import bass_rust
    cnt = 0
    for f in nc.m.functions:
        for blk in f.blocks:
            insts = blk.instructions
            out = []
            changed = False
            for inst in insts:
                if (type(inst).__name__ not in _WAIT_EXEMPT
                        and not isinstance(inst, bass_rust.InstISA)):
                    si = inst.sync_info
                    if si is not None and len(si.on_wait) > 1:
                        surplus = list(si.on_wait[:-1])
                        # each EventSemaphore carries at most 2 waits
                        for j in range(0, len(surplus), 2):
                            ev = bass_rust.InstEventSemaphore(name=f"I-wsplit-{cnt}")
                            cnt += 1
                            ev.engine = inst.engine
                            ev.bass_nofuse = True
                            ev.sync_info = bass_rust.SyncInfo(
                                on_wait=surplus[j:j + 2], on_update=[])
                            out.append(ev)
                        inst.sync_info = bass_rust.SyncInfo(
                            on_wait=[si.on_wait[-1]], on_update=list(si.on_update))
                        changed = True
                out.append(inst)
            if changed:
                blk.instructions = out


def _body(nc, tc, D, out_d, fl):
    import itertools
    _bc_ctr = itertools.count()
    ctx = ExitStack()

    # ---------- ambient pools ----------
    pv = ctx.enter_context(tc.tile_pool(name="pv", bufs=1))
    sm = ctx.enter_context(tc.tile_pool(name="sm", bufs=2))
    # Residual-chain ring: tags r0..r7, two generations in flight per tag.
    # Generation order per tag: x -> x1 -> O -> x2 -> Or -> x3.
    resid = ctx.enter_context(tc.tile_pool(name="resid", bufs=2))
    dscr = ctx.enter_context(tc.tile_pool(name="dscr", bufs=4, space="DRAM"))

    def bcast(dst_ap, src_ap, parts, tn, tag):
        """Broadcast a [1,tn] SBUF row to [parts,tn] via a DRAM round-trip
        (engines cannot read partition-stride-0 SBUF APs; DRAM DMAs can)."""
        scr = dscr.tile([1, tn], F32, tag=tag, name=f"scr_{tag}_{next(_bc_ctr)}")
        nc.sync.dma_start(out=scr[:], in_=src_ap)
        nc.sync.dma_start(out=dst_ap, in_=scr[0:1, :].broadcast_to((parts, tn)))

    def rtile(k, name):
        return resid.tile([P, T], F32, tag=f"r{k}", name=name)

    def vec_tile(name, rows):
        nt = rows // P
        t = pv.tile([P, nt], F32, tag=name, name=f"v_{name}")
        nc.sync.dma_start(out=t[:], in_=D[name].rearrange("(k p) o -> p (k o)", p=P))
        return t

    xs = []
    for k in range(HT):
        t = rtile(k, f"x_{k}")
        nc.sync.dma_start(out=_fr(t[:]), in_=D["xT"][k * P:(k + 1) * P, :])
        xs.append(t)

    V = {}
    for name, rows in [
        ("sg_b", S), ("A", S), ("Dp1", H), ("outp_b", H), ("a_bq", H),
        ("a_const", H), ("r_bq", H), ("r_const", H), ("mg_b", H),
        ("f_b1", 4 * H), ("f_b2", H),
        ("sln_g", H), ("sln_b", H), ("n1_g", H), ("n1_b", H),
        ("n2_g", H), ("n2_b", H), ("n3_g", H), ("n3_b", H),
    ]:
        V[name] = vec_tile(name, rows)
    for i, c in enumerate(COMP):
        V[f"c{i}_b1"] = vec_tile(f"c{i}_b1", c)
        V[f"c{i}_b2"] = vec_tile(f"c{i}_b2", H)

    ones_col = pv.tile([P, 1], FR, tag="ones_col")
    nc.sync.dma_start(out=ones_col[:], in_=D["ones"][:, :])
    eps_t = pv.tile([1, 1], F32, tag="eps")
    nc.vector.memset(eps_t[:], EPS)
    ident = pv.tile([P, P], F32, tag="ident")
    make_identity(nc, ident[:])

    # ---------- helpers ----------
    def mm(ps, steps, nch=NCH):
        """ps[M,N] = sum_k steps[k].lhsT.T @ steps[k].rhs ; chunks the moving dim."""
        n = ps.shape[-1]
        K = len(steps)
        for c0 in range(0, n, nch):
            ce = min(c0 + nch, n)
            for k, (lt, rt) in enumerate(steps):
                nc.tensor.matmul(ps[:, c0:ce], _fr(lt), _fr(rt[:, c0:ce]),
                                 start=(k == 0), stop=(k == K - 1))

    def load_wblocks(pool, dram_ap, nk, cols, tag, c0=0, bufs=1):
        """Load nk row-blocks [P, cols] of a pre-transposed weight, cols [c0, c0+cols)."""
        tiles = []
        for k in range(nk):
            t = pool.tile([P, cols], FR, tag=f"{tag}{k}", bufs=bufs,
                          name=f"{tag}{k}_{c0}")
            nc.sync.dma_start(out=t[:], in_=dram_ap[k * P:(k + 1) * P, c0:c0 + cols])
            tiles.append(t)
        return tiles

    def proj(wname, rhs_tiles, epilogue, pool, ppool, tag, nk=HT, mh=4, wbufs=2):
        """out[m] = epilogue(m, psum(W^T[:,m] @ rhs)), streaming W in col-halves.

        mh: m-tiles per column group (4 -> [P,512] blocks).
        """
        for half in range(HT // mh):
            wb = load_wblocks(pool, D[wname], nk, mh * P, tag, c0=half * mh * P,
                              bufs=wbufs)
            for ml in range(mh):
                m = half * mh + ml
                ps = ppool.tile([P, T], F32, tag="pbig", name=f"{tag}ps{m}")
                mm(ps, [(wb[k][:, ml * P:(ml + 1) * P], rhs_tiles[k][:])
                        for k in range(nk)])
                epilogue(m, ps)

    def layer_norm(z, gname, pools, mk_out, Tn=T, round_out=True):
        """Feature-dim (partition) LN. z: list of HT [P,Tn] tiles.
        mk_out(k) -> output tile. pools = (pp_stat, lnsq, pbc)."""
        pp_stat, lnsq, pbc = pools
        nchunk = max(1, Tn // NCH)
        cw = min(Tn, NCH)
        ps_s = [pp_stat.tile([1, cw], F32, tag="st", name=f"lnps_s{c}") for c in range(nchunk)]
        ps_q = [pp_stat.tile([1, cw], F32, tag="st", name=f"lnps_q{c}") for c in range(nchunk)]
        for c in range(nchunk):
            for k in range(HT):
                nc.tensor.matmul(ps_s[c][:, :], _fr(ones_col[:, 0:1]),
                                 _fr(z[k][:, c * cw:(c + 1) * cw]),
                                 start=(k == 0), stop=(k == HT - 1))
        for k in range(HT):
            sq = lnsq.tile([P, Tn], F32, tag="lnsq")
            nc.vector.tensor_mul(_fr(sq[:]), z[k][:], z[k][:])
            for c in range(nchunk):
                nc.tensor.matmul(ps_q[c][:, :], _fr(ones_col[:, 0:1]),
                                 _fr(sq[:, c * cw:(c + 1) * cw]),
                                 start=(k == 0), stop=(k == HT - 1))
        rstd = lnsq.tile([1, Tn], F32, tag="rstd", bufs=1, name="rstd")
        mr = lnsq.tile([1, Tn], F32, tag="mr", bufs=1, name="mr")
        for c in range(nchunk):
            cs = slice(c * cw, (c + 1) * cw)
            mean_c = lnsq.tile([1, cw], F32, tag="mean", bufs=2, name="mean_c")
            var_c = lnsq.tile([1, cw], F32, tag="var", bufs=2, name="var_c")
            nc.scalar.activation(mean_c[:], ps_s[c][:], AF.Copy, bias=0.0, scale=1.0 / H)
            nc.vector.tensor_mul(var_c[:], mean_c[:], mean_c[:])
            nc.vector.scalar_tensor_tensor(out=var_c[:], in0=ps_q[c][:], scalar=1.0 / H,
                                           in1=var_c[:], op0=OP.mult, op1=OP.subtract)
            nc.scalar.activation(var_c[:], var_c[:], AF.Sqrt, bias=eps_t[:, 0:1])
            nc.vector.reciprocal(rstd[:, cs], var_c[:])
            nc.vector.tensor_mul(mr[:, cs], mean_c[:], rstd[:, cs])
        bc_r = pbc.tile([P, Tn], F32, tag="bc", name="bc_r")
        bc_mr = pbc.tile([P, Tn], F32, tag="bc", name="bc_mr")
        bcast(bc_r[:], rstd[0:1, 0:Tn], P, Tn, "r")
        bcast(bc_mr[:], mr[0:1, 0:Tn], P, Tn, "mr")
        g_t, b_t = V[f"{gname}_g"], V[f"{gname}_b"]
        outs = []
        cast = _fr if round_out else (lambda a: a)
        for k in range(HT):
            o = mk_out(k)
            nc.vector.tensor_mul(cast(o[:]), z[k][:], bc_r[:])
            nc.vector.tensor_sub(cast(o[:]), o[:], bc_mr[:])
            if not fl[f"{gname}_trivial"]:
                nc.vector.tensor_scalar(out=cast(o[:]), in0=o[:],
                                        scalar1=g_t[:, k:k + 1],
                                        scalar2=b_t[:, k:k + 1], op0=OP.mult, op1=OP.add)
            outs.append(o)
        return outs

    # =========================================================================
    # x^T  (resid generation 1)
    # =========================================================================
    # =========================================================================
    # Stage A: SSM layer
    # =========================================================================
    with tc.tile_pool(name="ssm2", bufs=1) as ssm2, \
         tc.tile_pool(name="ppA", bufs=2, space="PSUM") as ppA:
        states = ssm2.tile([P, T], F32, tag="states")
        wC = ssm2.tile([S, H], FR, tag="wC")
        nc.sync.dma_start(out=wC[:], in_=D["wCT"][:, :])
        with tc.tile_pool(name="ssm1", bufs=1) as ssm1:
            wsg = load_wblocks(ssm1, D["wsgT"], HT, S, "wsg")
            wB = load_wblocks(ssm1, D["wBT"], HT, S, "wB")

            psG = ppA.tile([P, T], F32, tag="pbig")
            mm(psG, [(wsg[k][:], xs[k][:]) for k in range(HT)])
            gate = ssm1.tile([P, T], F32, tag="gate")
            nc.scalar.activation(gate[:], psG[:], AF.Sigmoid, bias=V["sg_b"][:, 0:1])

            psB = ppA.tile([P, T], F32, tag="pbig")
            mm(psB, [(wB[k][:], xs[k][:]) for k in range(HT)])
            u = ssm1.tile([P, T], F32, tag="u")
            nc.vector.tensor_mul(u[:], gate[:], psB[:])

            nc.vector.tensor_tensor_scan(_fr(states[:]),
                                         V["A"][:, 0:1].to_broadcast((P, T)), u[:],
                                         0.0, op0=OP.mult, op1=OP.add)

        with tc.tile_pool(name="lnzA", bufs=8) as lnz, \
             tc.tile_pool(name="lnsqA", bufs=3) as lnsq, \
             tc.tile_pool(name="bcA", bufs=2) as pbc, \
             tc.tile_pool(name="lnoA", bufs=8) as lnout, \
             tc.tile_pool(name="ppstA", bufs=4, space="PSUM") as ppst:
            z1 = []
            for m in range(HT):
                psY = ppA.tile([P, T], F32, tag="pbig", name=f"psY{m}")
                mm(psY, [(wC[:, m * P:(m + 1) * P], states[:])])
                zm = lnz.tile([P, T], F32, tag="z", name=f"z1_{m}")
                nc.vector.scalar_tensor_tensor(out=_fr(zm[:]), in0=xs[m][:],
                                               scalar=V["Dp1"][:, m:m + 1], in1=psY[:],
                                               op0=OP.mult, op1=OP.add)
                z1.append(zm)
            ln1 = layer_norm(z1, "sln", (ppst, lnsq, pbc),
                             lambda k: lnout.tile([P, T], F32, tag="ln1", name=f"ln1_{k}"))

            with tc.tile_pool(name="wouA", bufs=1) as wpo:
                z2 = []

                def ep_outp(m, ps):
                    zm = lnz.tile([P, T], F32, tag="z", name=f"z2_{m}")
                    if fl["outp_b_zero"]:
                        nc.vector.tensor_add(_fr(zm[:]), ps[:], xs[m][:])
                    else:
                        nc.vector.scalar_tensor_tensor(out=_fr(zm[:]), in0=ps[:],
                                                       scalar=V["outp_b"][:, m:m + 1],
                                                       in1=xs[m][:], op0=OP.add, op1=OP.add)
                    z2.append(zm)

                proj("outp_wT", ln1, ep_outp, wpo, ppA, "wou", wbufs=1)
            # x1 = resid generation 2
            x1 = layer_norm(z2, "n1", (ppst, lnsq, pbc),
                            lambda k: rtile(k, f"x1_{k}"))

    # =========================================================================
    # Stage B: self-attention
    # =========================================================================
    Oh = [rtile(g, f"oh{g}") for g in range(HT)]  # resid generation 3
    with tc.tile_pool(name="pQ", bufs=1) as pQ, \
         tc.tile_pool(name="pK", bufs=1) as pK, \
         tc.tile_pool(name="pV", bufs=1) as pV:
        with tc.tile_pool(name="wqkv", bufs=1) as wqkv, \
             tc.tile_pool(name="ppB1", bufs=2, space="PSUM") as ppB1:
            Qh, Kh, Vp = [], [], []

            def ep_q(m, ps):
                qm = pQ.tile([P, T], F32, tag=f"q{m}", name=f"q{m}")
                if fl["a_bq_zero"]:
                    nc.vector.tensor_copy(out=_fr(qm[:]), in_=ps[:])
                else:
                    nc.vector.tensor_scalar_add(_fr(qm[:]), ps[:], V["a_bq"][:, m:m + 1])
                Qh.append(qm)

            def ep_k(m, ps):
                km = pK.tile([P, T], F32, tag=f"k{m}", name=f"k{m}")
                nc.vector.tensor_copy(out=_fr(km[:]), in_=ps[:])
                Kh.append(km)

            proj("a_wqT", x1, ep_q, wqkv, ppB1, "wq", wbufs=1)
            proj("a_wkT", x1, ep_k, wqkv, ppB1, "wq", wbufs=1)
            # V token-major, with a ones column appended per head
            for kt in range(HT):
                vt = pV.tile([P, NH * (DH + 1)], FR, tag=f"v{kt}", name=f"v{kt}")
                nc.sync.dma_start(
                    out=vt[:].rearrange("p (h c) -> p h c", c=DH + 1)[:, :, DH:DH + 1],
                    in_=D["ones16"].rearrange("p (h o) -> p h o", o=1))
                Vp.append(vt)
            for vh in range(2):
                wvb = load_wblocks(wqkv, D["a_wvT"], HT, 512, "wq", c0=vh * 512)
                hs = 512 // (DH + 1) + 1  # 8 heads per half
                for kt in range(HT):
                    ps = ppB1.tile([P, 512], F32, tag="pvh", name=f"psV{vh}_{kt}")
                    mm(ps, [(x1[k][:, kt * P:(kt + 1) * P], wvb[k][:])
                            for k in range(HT)])
                    nc.vector.tensor_copy(
                        out=Vp[kt][:].rearrange("p (h c) -> p h c", c=DH + 1)[
                            :, 8 * vh:8 * (vh + 1), 0:DH],
                        in_=ps[:].rearrange("p (h c) -> p h c", c=DH)[:, :, :])

        with tc.tile_pool(name="pexp", bufs=4) as pexp, \
             tc.tile_pool(name="pbcB", bufs=2) as pbcB, \
             tc.tile_pool(name="ppSC", bufs=2, space="PSUM") as ppSC, \
             tc.tile_pool(name="ppAV", bufs=2, space="PSUM") as ppAV:
            for h in range(NH):
                g, ho = h // 2, (h % 2) * DH
                psA = ppAV.tile([DH + 1, T], F32, tag="pav", name=f"pav{h}")
                for kt in range(HT):
                    psS = ppSC.tile([P, T], F32, tag="psc", name=f"psc{h}_{kt}")
                    mm(psS, [(Kh[g][ho:ho + DH, kt * P:(kt + 1) * P],
                              Qh[g][ho:ho + DH, :])])
                    et = pexp.tile([P, T], FR, tag="exp", name=f"exp{h}_{kt}")
                    nc.scalar.activation(et[:], psS[:], AF.Exp, scale=1.0 / 8.0)
                    vslice = Vp[kt][:, h * (DH + 1):(h + 1) * (DH + 1)]
                    for c0 in range(0, T, NCH):
                        nc.tensor.matmul(psA[:, c0:c0 + NCH], _fr(vslice),
                                         _fr(et[:, c0:c0 + NCH]),
                                         start=(kt == 0), stop=(kt == HT - 1))
                rcp = sm.tile([1, T], F32, tag="stat", name=f"rcp{h}")
                nc.vector.reciprocal(rcp[:], psA[DH:DH + 1, :])
                rb = pbcB.tile([DH, T], F32, tag="rb", name=f"rb{h}")
                bcast(rb[:], rcp[0:1, :], DH, T, "rc")
                nc.vector.tensor_mul(_fr(Oh[g][ho:ho + DH, :]), psA[0:DH, :], rb[:])

    with tc.tile_pool(name="lnzB", bufs=8) as lnz, \
         tc.tile_pool(name="lnsqB", bufs=3) as lnsq, \
         tc.tile_pool(name="bcB2", bufs=2) as pbc, \
         tc.tile_pool(name="woB", bufs=1) as wpo, \
         tc.tile_pool(name="ppB3", bufs=2, space="PSUM") as ppB3, \
         tc.tile_pool(name="ppstB", bufs=4, space="PSUM") as ppst:
        z3 = []

        def ep_wo(m, ps):
            zm = lnz.tile([P, T], F32, tag="z", name=f"z3_{m}")
            if fl["a_const_zero"]:
                nc.vector.tensor_add(_fr(zm[:]), ps[:], x1[m][:])
            else:
                nc.vector.scalar_tensor_tensor(out=_fr(zm[:]), in0=ps[:],
                                               scalar=V["a_const"][:, m:m + 1],
                                               in1=x1[m][:], op0=OP.add, op1=OP.add)
            z3.append(zm)

        proj("a_woT", Oh, ep_wo, wpo, ppB3, "wo")
        # x2 = resid generation 4
        x2 = layer_norm(z3, "n2", (ppst, lnsq, pbc),
                        lambda k: rtile(k, f"x2_{k}"))

    # =========================================================================
    # Stage C: hierarchical memory retrieval + gated merge
    # =========================================================================
    cstk = ExitStack()
    with cstk:
        pKr = cstk.enter_context(tc.tile_pool(name="pKr", bufs=1))
        pVr = cstk.enter_context(tc.tile_pool(name="pVr", bufs=1))
        Kr, Vr = {}, {}
        with tc.tile_pool(name="pc", bufs=1) as pc:
            chat = {}
            with tc.tile_pool(name="cw", bufs=1) as cw, \
                 tc.tile_pool(name="cmid", bufs=1) as cmid, \
                 tc.tile_pool(name="ppC1", bufs=6, space="PSUM") as ppC1:
                for i, cwid in enumerate(COMP):
                    ct = cwid // P
                    with tc.tile_pool(name=f"pmT{i}", bufs=1) as pmT:
                        mT = load_wblocks(pmT, D[f"m{i}T"], HT, 256, "mT")
                        w1 = load_wblocks(cw, D[f"c{i}_w1T"], HT, cwid, "cwx")
                        mid = []
                        for cm in range(ct):
                            ps = ppC1.tile([P, 256], F32, tag="pc1", name=f"pm{i}_{cm}")
                            mm(ps, [(w1[k][:, cm * P:(cm + 1) * P], mT[k][:])
                                    for k in range(HT)])
                            md = cmid.tile([P, 256], FR, tag=f"mid{cm}", name=f"mid{i}_{cm}")
                            nc.scalar.activation(md[:], ps[:], AF.Relu,
                                                 bias=V[f"c{i}_b1"][:, cm:cm + 1])
                            mid.append(md)
                        w2 = load_wblocks(cw, D[f"c{i}_w2T"], ct, H, "cwx")
                        for m in range(HT):
                            ps = ppC1.tile([P, 256], F32, tag="pc1", name=f"pc{i}_{m}")
                            mm(ps, [(w2[k][:, m * P:(m + 1) * P], mid[k][:])
                                    for k in range(ct)])
                            cm_t = pc.tile([P, 256], F32, tag=f"c{i}_{m}", name=f"c{i}_{m}")
                            if fl[f"c{i}_b2_zero"]:
                                nc.vector.tensor_copy(out=_fr(cm_t[:]), in_=ps[:])
                            else:
                                nc.vector.tensor_scalar_add(_fr(cm_t[:]), ps[:],
                                                            V[f"c{i}_b2"][:, m:m + 1])
                            chat.setdefault(i, []).append(cm_t)
            with tc.tile_pool(name="rkv", bufs=1) as rkv, \
                 tc.tile_pool(name="ppC2", bufs=2, space="PSUM") as ppC2:
                wkr = load_wblocks(rkv, D["r_wkT"], HT, H, "rkv")
                for i in range(3):
                    Kr[i] = []
                    for m in range(HT):
                        ps = ppC2.tile([P, 256], F32, tag="pkv", name=f"pk{i}_{m}")
                        mm(ps, [(wkr[k][:, m * P:(m + 1) * P], chat[i][k][:])
                                for k in range(HT)])
                        kt_ = pKr.tile([P, 256], F32, tag=f"kr{i}_{m}", name=f"kr{i}_{m}")
                        nc.vector.tensor_copy(out=_fr(kt_[:]), in_=ps[:])
                        Kr[i].append(kt_)
                wvr = load_wblocks(rkv, D["r_wvT"], HT, H, "rkv")
                for i in range(3):
                    Vr[i] = []
                    for kvt in range(2):
                        ps = ppC2.tile([P, T], F32, tag="pkv2", name=f"pv{i}_{kvt}")
                        mm(ps, [(chat[i][k][:, kvt * P:(kvt + 1) * P], wvr[k][:])
                                for k in range(HT)])
                        vt = pVr.tile([P, T], F32, tag=f"vr{i}_{kvt}", name=f"vr{i}_{kvt}")
                        nc.vector.tensor_copy(out=_fr(vt[:]), in_=ps[:])
                        Vr[i].append(vt)
        pQr = cstk.enter_context(tc.tile_pool(name="pQr", bufs=1))
        with tc.tile_pool(name="rwq", bufs=1) as rwq, \
             tc.tile_pool(name="ppC3", bufs=3, space="PSUM") as ppC3:
            Qr = []

            def ep_qr(m, ps):
                qm = pQr.tile([P, T], F32, tag=f"qr{m}", name=f"qr{m}")
                if fl["r_bq_zero"]:
                    nc.vector.tensor_copy(out=_fr(qm[:]), in_=ps[:])
                else:
                    nc.vector.tensor_scalar_add(_fr(qm[:]), ps[:], V["r_bq"][:, m:m + 1])
                Qr.append(qm)

            proj("r_wqT", x2, ep_qr, rwq, ppC3, "rwq")

        Or = [rtile(m, f"orr{m}") for m in range(HT)]  # resid generation 5
        with tc.tile_pool(name="pexpR", bufs=4) as pexpR, \
             tc.tile_pool(name="ptwR", bufs=2) as ptw, \
             tc.tile_pool(name="pbcR", bufs=3) as pbcR, \
             tc.tile_pool(name="ppSCr", bufs=1, space="PSUM") as ppSCr, \
             tc.tile_pool(name="ppAVr", bufs=2, space="PSUM") as ppAVr, \
             tc.tile_pool(name="ppsum", bufs=2, space="PSUM") as ppsum:
            for i in range(3):
                for r in range(RH):
                    ets = []
                    for kvt in range(2):
                        psS = ppSCr.tile([P, T], F32, tag="psc", name=f"rsc{i}{r}{kvt}")
                        mm(psS, [(Kr[i][2 * r + kc][:, kvt * P:(kvt + 1) * P],
                                  Qr[2 * r + kc][:]) for kc in range(2)])
                        et = pexpR.tile([P, T], FR, tag="expr", name=f"re{i}{r}{kvt}")
                        nc.scalar.activation(et[:], psS[:], AF.Exp, scale=1.0 / 16.0)
                        ets.append(et)
                    rcp = sm.tile([1, T], F32, tag="stat", name=f"rcpr{i}{r}")
                    for c0 in range(0, T, NCH):
                        psZ = ppsum.tile([1, NCH], F32, tag="pz", name=f"rz{i}{r}{c0}")
                        for kvt in range(2):
                            nc.tensor.matmul(psZ[:, :], _fr(ones_col[:, 0:1]),
                                             _fr(ets[kvt][:, c0:c0 + NCH]),
                                             start=(kvt == 0), stop=(kvt == 1))
                        nc.vector.reciprocal(rcp[:, c0:c0 + NCH], psZ[:, :])
                    rb = pbcR.tile([P, T], F32, tag="rbr", name=f"rbr{i}{r}")
                    bcast(rb[:], rcp[0:1, :], P, T, "rr")
                    for md in range(2):
                        psA = ppAVr.tile([P, T], F32, tag="pav", name=f"rav{i}{r}{md}")
                        col = RDH * r + P * md
                        for c0 in range(0, T, NCH):
                            for kvt in range(2):
                                nc.tensor.matmul(psA[:, c0:c0 + NCH],
                                                 _fr(Vr[i][kvt][:, col:col + P]),
                                                 _fr(ets[kvt][:, c0:c0 + NCH]),
                                                 start=(kvt == 0), stop=(kvt == 1))
                        dst = Or[2 * r + md]
                        if i == 0:
                            nc.vector.tensor_mul(_fr(dst[:]), psA[:], rb[:])
                        else:
                            tw = ptw.tile([P, T], F32, tag="tw", name=f"tw{i}{r}{md}")
                            nc.vector.tensor_mul(tw[:], psA[:], rb[:])
                            nc.vector.tensor_add(_fr(dst[:]), dst[:], tw[:])

    with tc.tile_pool(name="pcomb", bufs=1) as pcomb:
        with tc.tile_pool(name="rwo", bufs=1) as rwo, \
             tc.tile_pool(name="ppC5", bufs=3, space="PSUM") as ppC5:
            comb = []

            def ep_ro(m, ps):
                cm_ = pcomb.tile([P, T], F32, tag=f"cb{m}", name=f"cb{m}")
                if fl["r_const_zero"]:
                    nc.scalar.activation(_fr(cm_[:]), ps[:], AF.Copy, bias=0.0, scale=1.0 / 3.0)
                else:
                    nc.vector.tensor_scalar(out=_fr(cm_[:]), in0=ps[:], scalar1=1.0 / 3.0,
                                            scalar2=V["r_const"][:, m:m + 1],
                                            op0=OP.mult, op1=OP.add)
                comb.append(cm_)

            proj("r_woT", Or, ep_ro, rwo, ppC5, "rwo")

        x3 = []
        with tc.tile_pool(name="mgw", bufs=1) as mgw, \
             tc.tile_pool(name="pgw", bufs=2) as pgw, \
             tc.tile_pool(name="ptmp", bufs=2) as ptmp, \
             tc.tile_pool(name="ppC6", bufs=3, space="PSUM") as ppC6:
            for half in range(2):
                wb = load_wblocks(mgw, D["mg_wT"], 2 * HT, 4 * P, "mg", c0=half * 4 * P,
                                  bufs=2)
                for ml in range(4):
                    m = half * 4 + ml
                    ps = ppC6.tile([P, T], F32, tag="pbig", name=f"mgps{m}")
                    steps = [(wb[k][:, ml * P:(ml + 1) * P], x2[k][:]) for k in range(HT)]
                    steps += [(wb[HT + k][:, ml * P:(ml + 1) * P], comb[k][:])
                              for k in range(HT)]
                    mm(ps, steps)
                    gw = pgw.tile([P, T], F32, tag="gw", name=f"gw{m}")
                    nc.scalar.activation(gw[:], ps[:], AF.Sigmoid, bias=V["mg_b"][:, m:m + 1])
                    d = ptmp.tile([P, T], F32, tag="d", name=f"d{m}")
                    nc.vector.tensor_sub(d[:], x2[m][:], comb[m][:])
                    nc.vector.tensor_mul(d[:], gw[:], d[:])
                    s = ptmp.tile([P, T], F32, tag="s", name=f"s{m}")
                    nc.vector.tensor_add(s[:], x2[m][:], comb[m][:])
                    xm = rtile(m, f"x3_{m}")  # resid generation 6
                    nc.vector.tensor_add(_fr(xm[:]), s[:], d[:])
                    x3.append(xm)

    # =========================================================================
    # Stage D: FFN in token-halves + final LN + transpose to [T, H]
    # =========================================================================
    TH = T // 2
    with tc.tile_pool(name="ph", bufs=1) as ph:
        hts = {0: [], 1: []}
        with tc.tile_pool(name="fw1", bufs=2) as fw1, \
             tc.tile_pool(name="ppD1", bufs=4, space="PSUM") as ppD1:
            for mg_i in range(8):
                wblk = load_wblocks(fw1, D["f_w1T"], HT, 512, "w1s",
                                    c0=mg_i * 512, bufs=2)
                for th in range(2):
                    c0 = th * TH
                    for ml in range(4):
                        m_abs = mg_i * 4 + ml
                        ps = ppD1.tile([P, TH], F32, tag="p1", name=f"f1ps{th}_{m_abs}")
                        mm(ps, [(wblk[k][:, ml * P:(ml + 1) * P],
                                 x3[k][:, c0:c0 + TH]) for k in range(HT)])
                        htile = ph.tile([P, TH], BF16, tag=f"h{th}_{m_abs}",
                                        name=f"h{th}_{m_abs}")
                        nc.scalar.activation(htile[:], ps[:], AF.Gelu,
                                             bias=V["f_b1"][:, m_abs:m_abs + 1])
                        hts[th].append(htile)
        for th in range(2):
            c0 = th * TH
            with tc.tile_pool(name="lnzD", bufs=8) as lnz:
                z4 = []
                with tc.tile_pool(name="fw2", bufs=4) as fw2, \
                     tc.tile_pool(name="ppD2", bufs=1, space="PSUM") as ppD2:
                    pso = [ppD2.tile([P, TH], F32, tag=f"p2_{m}", name=f"pso{th}_{m}")
                           for m in range(HT)]
                    for k2 in range(4 * HT):
                        wt = fw2.tile([P, H], BF16, tag="w2s", name=f"w2s{th}_{k2}")
                        nc.sync.dma_start(out=wt[:], in_=D["f_w2T"][k2 * P:(k2 + 1) * P, :])
                        for mo in range(HT):
                            nc.tensor.matmul(pso[mo][:, :],
                                             wt[:, mo * P:(mo + 1) * P],
                                             hts[th][k2][:],
                                             start=(k2 == 0), stop=(k2 == 4 * HT - 1))
                    for mo in range(HT):
                        zm = lnz.tile([P, TH], F32, tag="z", name=f"z4_{th}_{mo}")
                        if fl["f_b2_zero"]:
                            nc.vector.tensor_add(_fr(zm[:]), pso[mo][:], x3[mo][:, c0:c0 + TH])
                        else:
                            nc.vector.scalar_tensor_tensor(out=_fr(zm[:]), in0=pso[mo][:],
                                                           scalar=V["f_b2"][:, mo:mo + 1],
                                                           in1=x3[mo][:, c0:c0 + TH],
                                                           op0=OP.add, op1=OP.add)
                        z4.append(zm)
                with tc.tile_pool(name="lnsqD", bufs=2) as lnsq, \
                     tc.tile_pool(name="bcD", bufs=2) as pbc, \
                     tc.tile_pool(name="lnoD", bufs=8) as lnout, \
                     tc.tile_pool(name="stg", bufs=2) as stg_pool, \
                     tc.tile_pool(name="ppstD", bufs=4, space="PSUM") as ppst, \
                     tc.tile_pool(name="ppT", bufs=4, space="PSUM") as ppT:
                    fin = layer_norm(z4, "n3", (ppst, lnsq, pbc),
                                     lambda k: lnout.tile([P, TH], F32, tag="fin",
                                                          name=f"fin{th}_{k}"),
                                     Tn=TH, round_out=False)
                    for tt in range(TH // P):
                        stg = stg_pool.tile([P, H], F32, tag="stg", name=f"stg{th}_{tt}")
                        for k2 in range(HT):
                            psT = ppT.tile([P, P], F32, tag="pt", name=f"pT{th}_{tt}_{k2}")
                            nc.tensor.transpose(psT[:, :],
                                                fin[k2][:, tt * P:(tt + 1) * P],
                                                ident[:])
                            nc.vector.tensor_copy(out=stg[:, k2 * P:(k2 + 1) * P],
                                                  in_=psT[:, :])
                        row0 = c0 + tt * P
                        nc.sync.dma_start(out=out_d[row0:row0 + P, :], in_=stg[:])
    ctx.close()


# =============================================================================
# Host side
# =============================================================================
_CACHE = {}


def _flags(g):
    def zero(a):
        return bool(np.all(a == 0.0))

    fl = {}
    for n in ("sln", "n1", "n2", "n3"):
        fl[f"{n}_trivial"] = bool(np.all(g[f"{n}_g"] == 1.0) and zero(g[f"{n}_b"]))
    fl["outp_b_zero"] = zero(g["outp_b"])
    wq_b, wk_b, wv_b = np.split(g["attn_in_b"], 3, 0)
    fl["a_bq_zero"] = zero(wq_b)
    a_const = wv_b @ g["attn_out_w"].T + g["attn_out_b"]
    fl["a_const_zero"] = zero(a_const)
    rq_b, rk_b, rv_b = np.split(g["retr_in_b"], 3, 0)
    fl["r_bq_zero"] = zero(rq_b)
    r_const = rv_b @ g["retr_out_w"].T + g["retr_out_b"]
    fl["r_const_zero"] = zero(r_const)
    for i in range(3):
        fl[f"c{i}_b2_zero"] = zero(g[f"c{i}_b2"])
    fl["f_b2_zero"] = zero(g["ffn_b2"])
    return fl, a_const, r_const


def kernel(**inputs):
    g = {k: np.ascontiguousarray(np.asarray(v, dtype=np.float32)) for k, v in inputs.items()}
    fl, a_const, r_const = _flags(g)

    key = tuple(sorted(fl.items()))
    if key not in _CACHE:
        _CACHE[key] = build_nc(fl)
    nc = _CACHE[key]

    def tr(a):
        return np.ascontiguousarray(a.T)

    wq, wk, wv = np.split(g["attn_in_w"], 3, 0)
    rq, rk, rv = np.split(g["retr_in_w"], 3, 0)
    shared = {
        "wsgT": tr(g["sgate_w"]), "wBT": tr(g["B_w"]), "wCT": tr(g["C_w"]),
        "A": np.exp(g["A_log"]).reshape(S, 1), "sg_b": g["sgate_b"].reshape(S, 1),
        "Dp1": (g["D"] + 1.0).reshape(H, 1),
        "outp_wT": tr(g["outp_w"]), "outp_b": g["outp_b"].reshape(H, 1),
        "a_wqT": tr(wq), "a_wkT": tr(wk), "a_wvT": tr(wv),
        "a_bq": np.split(g["attn_in_b"], 3, 0)[0].reshape(H, 1),
        "a_woT": tr(g["attn_out_w"]), "a_const": a_const.reshape(H, 1),
        "r_wqT": tr(rq), "r_wkT": tr(rk), "r_wvT": tr(rv),
        "r_bq": np.split(g["retr_in_b"], 3, 0)[0].reshape(H, 1),
        "r_woT": tr(g["retr_out_w"]), "r_const": r_const.reshape(H, 1),
        "mg_wT": tr(g["mg_w"]), "mg_b": g["mg_b"].reshape(H, 1),
        "f_w1T": tr(g["ffn_w1"]), "f_b1": g["ffn_b1"].reshape(4 * H, 1),
        "f_b2": g["ffn_b2"].reshape(H, 1),
    }
    for i in range(3):
        shared[f"c{i}_w1T"] = tr(g[f"c{i}_w1"])
        shared[f"c{i}_b1"] = g[f"c{i}_b1"].reshape(COMP[i], 1)
        shared[f"c{i}_w2T"] = tr(g[f"c{i}_w2"])
        shared[f"c{i}_b2"] = g[f"c{i}_b2"].reshape(H, 1)
    for n in ("sln", "n1", "n2", "n3"):
        shared[f"{n}_g"] = g[f"{n}_g"].reshape(H, 1)
        shared[f"{n}_b"] = g[f"{n}_b"].reshape(H, 1)
    shared["ones"] = np.ones((P, 1), np.float32)
    shared["ones16"] = np.ones((P, NH), np.float32)
    shared = {k: np.ascontiguousarray(v.astype(np.float32)) for k, v in shared.items()}
    import ml_dtypes
    shared["f_w2T"] = np.ascontiguousarray(tr(g["ffn_w2"]).astype(ml_dtypes.bfloat16))

    in_maps = []
    for b in range(B):
        m = dict(shared)
        m["xT"] = tr(g["x"][b])
        for i in range(3):
            m[f"m{i}T"] = tr(g[f"mem{i}"][b, -256:, :])
        in_maps.append(m)

    trace = os.environ.get("KERNEL_TRACE", "0") == "1"
    res = bass_utils.run_bass_kernel_spmd(nc, in_maps, core_ids=list(range(B)),
                                          trace=trace)
    global LAST_RESULTS
    LAST_RESULTS = res
    out = np.stack([res.results[b]["out"] for b in range(B)], axis=0)
    return out


LAST_RESULTS = None


def bench(n_iter=6, **inputs):
    """Time the on-device execution with device-resident inputs (excludes
    host->device transfer). Returns (best_seconds, out)."""
    import time

    import jax
    import jax.numpy as jnp
    from jax.sharding import Mesh, PartitionSpec
    from jax.experimental.shard_map import shard_map
    from concourse import bass2jax

    g = {k: np.ascontiguousarray(np.asarray(v, dtype=np.float32)) for k, v in inputs.items()}
    fl, a_const, r_const = _flags(g)
    key = tuple(sorted(fl.items()))
    if key not in _CACHE:
        _CACHE[key] = build_nc(fl)
    nc = _CACHE[key]
    in_maps = _in_maps(g, a_const, r_const)

    bass2jax.install_neuronx_cc_hook()
    import concourse.mybir as mybir_
    in_names, out_names, out_avals, zero_outs = [], [], [], []
    for alloc in nc.m.functions[0].allocations:
        if not isinstance(alloc, mybir_.MemoryLocationSet):
            continue
        name = alloc.memorylocations[0].name
        pid_name = nc.partition_id_tensor.name if nc.partition_id_tensor else None
        if alloc.kind == "ExternalInput":
            if name != pid_name:
                in_names.append(name)
        elif alloc.kind == "ExternalOutput":
            out_names.append(name)
            np_dt = mybir_.dt.np(alloc.dtype)
            out_avals.append(jax.core.ShapedArray(tuple(alloc.tensor_shape), np_dt))
            zero_outs.append(np.zeros(tuple(alloc.tensor_shape), np_dt))
    n_params = len(in_names)
    all_names = in_names + out_names
    if nc.partition_id_tensor is not None:
        all_names = all_names + [nc.partition_id_tensor.name]

    def _body(*args):
        operands = list(args)
        if nc.partition_id_tensor is not None:
            operands.append(bass2jax.partition_id_tensor())
        outs = bass2jax._bass_exec_p.bind(
            *operands, out_avals=tuple(out_avals), in_names=tuple(all_names),
            out_names=tuple(out_names), lowering_input_output_aliases=(),
            sim_require_finite=True, sim_require_nnan=True, nc=nc)
        return tuple(outs)

    devices = jax.devices()[:B]
    mesh = Mesh(np.asarray(devices), ("core",))
    nin = n_params + len(out_names)
    sharded = jax.jit(shard_map(_body, mesh=mesh,
                                in_specs=(PartitionSpec("core"),) * nin,
                                out_specs=(PartitionSpec("core"),) * len(out_names),
                                check_rep=False))
    sh = jax.sharding.NamedSharding(mesh, PartitionSpec("core"))
    concat_in = [np.concatenate([np.asarray(in_maps[c][i_name])
                                 for c in range(B)], axis=0) for i_name in in_names]
    concat_zeros = [np.zeros((B * z.shape[0], *z.shape[1:]), z.dtype) for z in zero_outs]
    dev_in = [jax.device_put(a, sh) for a in concat_in + concat_zeros]
    jax.block_until_ready(dev_in)
    best = None
    out = None
    for it in range(n_iter):
        t0 = time.perf_counter()
        out = sharded(*dev_in)
        jax.block_until_ready(out)
        dt = time.perf_counter() - t0
        print(f"  iter {it}: {dt * 1e3:.2f} ms")
        if best is None or dt < best:
            best = dt
    # pure-jax dispatch floor on the same mesh
    tiny = jax.jit(lambda a: a + 1.0)
    ta = jax.device_put(np.zeros((8, 128), np.float32),
                        jax.sharding.NamedSharding(mesh, PartitionSpec("core")))
    jax.block_until_ready(tiny(ta))
    t0 = time.perf_counter()
    for _ in range(8):
        jax.block_until_ready(tiny(ta))
    print(f"  tiny-op sync floor: {(time.perf_counter() - t0) / 8 * 1e3:.2f} ms")
    t0 = time.perf_counter()
    outs = [tiny(ta) for _ in range(16)]
    jax.block_until_ready(outs)
    print(f"  tiny-op queued floor: {(time.perf_counter() - t0) / 16 * 1e3:.2f} ms")
    # amortized: pipeline several calls to hide RPC latency
    nq = 16
    t0 = time.perf_counter()
    outs = [sharded(*dev_in) for _ in range(nq)]
    jax.block_until_ready(outs)
    amort = (time.perf_counter() - t0) / nq
    print(f"  amortized over {nq} queued calls: {amort * 1e3:.2f} ms")
    res = np.asarray(out[0]).reshape(B, T, H)
    return min(best, amort), res


def _in_maps(g, a_const, r_const):
    def tr(a):
        return np.ascontiguousarray(a.T)

    wq, wk, wv = np.split(g["attn_in_w"], 3, 0)
    rq, rk, rv = np.split(g["retr_in_w"], 3, 0)
    shared = {
        "wsgT": tr(g["sgate_w"]), "wBT": tr(g["B_w"]), "wCT": tr(g["C_w"]),
        "A": np.exp(g["A_log"]).reshape(S, 1), "sg_b": g["sgate_b"].reshape(S, 1),
        "Dp1": (g["D"] + 1.0).reshape(H, 1),
        "outp_wT": tr(g["outp_w"]), "outp_b": g["outp_b"].reshape(H, 1),
        "a_wqT": tr(wq), "a_wkT": tr(wk), "a_wvT": tr(wv),
        "a_bq": np.split(g["attn_in_b"], 3, 0)[0].reshape(H, 1),
        "a_woT": tr(g["attn_out_w"]), "a_const": a_const.reshape(H, 1),
        "r_wqT": tr(rq), "r_wkT": tr(rk), "r_wvT": tr(rv),
        "r_bq": np.split(g["retr_in_b"], 3, 0)[0].reshape(H, 1),
        "r_woT": tr(g["retr_out_w"]), "r_const": r_const.reshape(H, 1),
        "mg_wT": tr(g["mg_w"]), "mg_b": g["mg_b"].reshape(H, 1),
        "f_w1T": tr(g["ffn_w1"]), "f_b1": g["ffn_b1"].reshape(4 * H, 1),
        "f_b2": g["ffn_b2"].reshape(H, 1),
    }
    for i in range(3):
        shared[f"c{i}_w1T"] = tr(g[f"c{i}_w1"])
        shared[f"c{i}_b1"] = g[f"c{i}_b1"].reshape(COMP[i], 1)
        shared[f"c{i}_w2T"] = tr(g[f"c{i}_w2"])
        shared[f"c{i}_b2"] = g[f"c{i}_b2"].reshape(H, 1)
    for n in ("sln", "n1", "n2", "n3"):
        shared[f"{n}_g"] = g[f"{n}_g"].reshape(H, 1)
        shared[f"{n}_b"] = g[f"{n}_b"].reshape(H, 1)
    shared["ones"] = np.ones((P, 1), np.float32)
    shared["ones16"] = np.ones((P, NH), np.float32)
    shared = {k: np.ascontiguousarray(v.astype(np.float32)) for k, v in shared.items()}
    import ml_dtypes
    shared["f_w2T"] = np.ascontiguousarray(tr(g["ffn_w2"]).astype(ml_dtypes.bfloat16))
    in_maps = []
    for b in range(B):
        m = dict(shared)
        m["xT"] = tr(g["x"][b])
        for i in range(3):
            m[f"m{i}T"] = tr(g[f"mem{i}"][b, -256:, :])
        in_maps.append(m)
    return in_maps



# revision 3
# speedup vs baseline: 1.0136x; 1.0136x over previous
"""TRN2 Bass/Tile kernel for nn_DHSMBlock (SSM + self-attn + hierarchical memory + FFN).

Sharding: data-parallel over batch. B=8 rows -> 8 NeuronCores, one row per core,
no collectives. Each core gets the full weight set (host pre-transposed).

On-device layout is feature-major: every activation lives as X^T [feature, token]
so that all matmuls contract over the partition dim. Weights are shipped as W^T
[in_f, out_f] (host numpy transpose). LayerNorm is over the feature dim =
partition dim; stats are computed with ones-vector matmuls on the PE and
broadcast back with SBUF->SBUF stride-0 DMAs. The SSM recurrence is a single
DVE tensor_tensor_scan instruction per core. Softmax is computed k-major
(scores^T), so no transposes are needed anywhere except the final output.
"""

import os
from contextlib import ExitStack

import numpy as np

os.environ.setdefault("MYCRO_LOCAL_CACHE", "1")

import concourse.bass as bass
import concourse.mybir as mybir
import concourse.tile as tile
from concourse import bass_utils
from concourse.masks import make_identity

F32 = mybir.dt.float32
FR = mybir.dt.float32r
BF16 = mybir.dt.bfloat16
AF = mybir.ActivationFunctionType
OP = mybir.AluOpType

B, T, H, S = 8, 1024, 1024, 128
NH, DH = 16, 64          # self-attention heads
RH, RDH = 4, 256         # retriever heads
COMP = [1024, 512, 256]  # compressor widths
P = 128
HT = H // P              # 8 feature tiles
NCH = 512                # matmul moving-dim chunk (one fp32 PSUM bank)
EPS = 1e-5


def _fr(ap):
    return ap.bitcast(FR)


def build_nc(fl):
    """Build the Bass program. fl: dict of host-known triviality flags."""
    nc = bass.Bass("TRN2", target_bir_lowering=False, debug=False, num_devices=8)
    D = {}

    def din(name, shape, dt=F32):
        D[name] = nc.dram_tensor(name, list(shape), dt, kind="ExternalInput").ap()

    din("xT", (H, T), FR)
    for i in range(3):
        din(f"m{i}T", (H, 256), FR)
    din("wsgT", (H, S), FR); din("wBT", (H, S), FR); din("wCT", (S, H), FR)
    din("A", (S, 1)); din("sg_b", (S, 1)); din("Dp1", (H, 1))
    din("outp_wT", (H, H), FR); din("outp_b", (H, 1))
    din("a_wqT", (H, H), FR); din("a_wkT", (H, H), FR); din("a_wvT", (H, H), FR)
    din("a_bq", (H, 1)); din("a_woT", (H, H), FR); din("a_const", (H, 1))
    for i, c in enumerate(COMP):
        din(f"c{i}_w1T", (H, c), FR); din(f"c{i}_b1", (c, 1))
        din(f"c{i}_w2T", (c, H), FR); din(f"c{i}_b2", (H, 1))
    din("r_wqT", (H, H), FR); din("r_wkT", (H, H), FR); din("r_wvT", (H, H), FR)
    din("r_bq", (H, 1)); din("r_woT", (H, H), FR); din("r_const", (H, 1))
    din("mg_wT", (2 * H, H), FR); din("mg_b", (H, 1))
    din("f_w1T", (H, 4 * H), FR); din("f_b1", (4 * H, 1))
    din("f_w2T", (4 * H, H), BF16); din("f_b2", (H, 1))
    for n in ("sln", "n1", "n2", "n3"):
        din(f"{n}_g", (H, 1)); din(f"{n}_b", (H, 1))
    din("ones", (P, 1), FR)
    din("ones16", (P, NH), FR)
    out_d = nc.dram_tensor("out", [T, H], F32, kind="ExternalOutput").ap()

    with tile.TileContext(nc, pool_alloc_mode="queue") as tc:
        _body(nc, tc, D, out_d, fl)
    _split_matmul_waits(nc)
    return nc


_WAIT_EXEMPT = {
    "InstEventSemaphore", "InstAllEngineBarrier",
    "InstUnconditionalBranch", "InstCompareAndBranch", "InstIndirectBranch",
    "InstHalt", "InstBranchHint",
}


def _split_matmul_waits(nc):
    """TPB engine instruction encodings carry at most one sync wait; move
    surplus waits onto a preceding same-engine no-op (sequencer WAITs)."""
    import bass_rust
    cnt = 0
    for f in nc.m.functions:
        for blk in f.blocks:
            insts = blk.instructions
            out = []
            changed = False
            for inst in insts:
                if (type(inst).__name__ not in _WAIT_EXEMPT
                        and not isinstance(inst, bass_rust.InstISA)):
                    si = inst.sync_info
                    if si is not None and len(si.on_wait) > 1:
                        surplus = list(si.on_wait[:-1])
                        # each EventSemaphore carries at most 2 waits
                        for j in range(0, len(surplus), 2):
                            ev = bass_rust.InstEventSemaphore(name=f"I-wsplit-{cnt}")
                            cnt += 1
                            ev.engine = inst.engine
                            ev.bass_nofuse = True
                            ev.sync_info = bass_rust.SyncInfo(
                                on_wait=surplus[j:j + 2], on_update=[])
                            out.append(ev)
                        inst.sync_info = bass_rust.SyncInfo(
                            on_wait=[si.on_wait[-1]], on_update=list(si.on_update))
                        changed = True
                out.append(inst)
            if changed:
                blk.instructions = out


def _body(nc, tc, D, out_d, fl):
    import itertools
    _bc_ctr = itertools.count()
    ctx = ExitStack()

    # ---------- ambient pools ----------
    pv = ctx.enter_context(tc.tile_pool(name="pv", bufs=1))
    sm = ctx.enter_context(tc.tile_pool(name="sm", bufs=2))
    # Residual-chain ring: tags r0..r7, two generations in flight per tag.
    # Generation order per tag: x -> x1 -> O -> x2 -> Or -> x3.
    resid = ctx.enter_context(tc.tile_pool(name="resid", bufs=2))
    dscr = ctx.enter_context(tc.tile_pool(name="dscr", bufs=4, space="DRAM"))

    def bcast(dst_ap, src_ap, parts, tn, tag):
        """Broadcast a [1,tn] SBUF row to [parts,tn] via a DRAM round-trip
        (engines cannot read partition-stride-0 SBUF APs; DRAM DMAs can)."""
        scr = dscr.tile([1, tn], F32, tag=tag, name=f"scr_{tag}_{next(_bc_ctr)}")
        nc.sync.dma_start(out=scr[:], in_=src_ap)
        nc.sync.dma_start(out=dst_ap, in_=scr[0:1, :].broadcast_to((parts, tn)))

    def rtile(k, name):
        return resid.tile([P, T], F32, tag=f"r{k}", name=name)

    def vec_tile(name, rows):
        nt = rows // P
        t = pv.tile([P, nt], F32, tag=name, name=f"v_{name}")
        nc.sync.dma_start(out=t[:], in_=D[name].rearrange("(k p) o -> p (k o)", p=P))
        return t

    xs = []
    for k in range(HT):
        t = rtile(k, f"x_{k}")
        nc.sync.dma_start(out=_fr(t[:]), in_=D["xT"][k * P:(k + 1) * P, :])
        xs.append(t)

    V = {}
    for name, rows in [
        ("sg_b", S), ("A", S), ("Dp1", H), ("outp_b", H), ("a_bq", H),
        ("a_const", H), ("r_bq", H), ("r_const", H), ("mg_b", H),
        ("f_b1", 4 * H), ("f_b2", H),
        ("sln_g", H), ("sln_b", H), ("n1_g", H), ("n1_b", H),
        ("n2_g", H), ("n2_b", H), ("n3_g", H), ("n3_b", H),
    ]:
        V[name] = vec_tile(name, rows)
    for i, c in enumerate(COMP):
        V[f"c{i}_b1"] = vec_tile(f"c{i}_b1", c)
        V[f"c{i}_b2"] = vec_tile(f"c{i}_b2", H)

    ones_col = pv.tile([P, 1], FR, tag="ones_col")
    nc.sync.dma_start(out=ones_col[:], in_=D["ones"][:, :])
    eps_t = pv.tile([1, 1], F32, tag="eps")
    nc.vector.memset(eps_t[:], EPS)
    ident = pv.tile([P, P], F32, tag="ident")
    make_identity(nc, ident[:])

    # ---------- helpers ----------
    def mm(ps, steps, nch=NCH):
        """ps[M,N] = sum_k steps[k].lhsT.T @ steps[k].rhs ; chunks the moving dim."""
        n = ps.shape[-1]
        K = len(steps)
        for c0 in range(0, n, nch):
            ce = min(c0 + nch, n)
            for k, (lt, rt) in enumerate(steps):
                nc.tensor.matmul(ps[:, c0:ce], _fr(lt), _fr(rt[:, c0:ce]),
                                 start=(k == 0), stop=(k == K - 1))

    def load_wblocks(pool, dram_ap, nk, cols, tag, c0=0, bufs=1):
        """Load nk row-blocks [P, cols] of a pre-transposed weight, cols [c0, c0+cols)."""
        tiles = []
        for k in range(nk):
            t = pool.tile([P, cols], FR, tag=f"{tag}{k}", bufs=bufs,
                          name=f"{tag}{k}_{c0}")
            nc.sync.dma_start(out=t[:], in_=dram_ap[k * P:(k + 1) * P, c0:c0 + cols])
            tiles.append(t)
        return tiles

    def proj(wname, rhs_tiles, epilogue, pool, ppool, tag, nk=HT, mh=4, wbufs=2):
        """out[m] = epilogue(m, psum(W^T[:,m] @ rhs)), streaming W in col-halves.

        mh: m-tiles per column group (4 -> [P,512] blocks).
        """
        for half in range(HT // mh):
            wb = load_wblocks(pool, D[wname], nk, mh * P, tag, c0=half * mh * P,
                              bufs=wbufs)
            for ml in range(mh):
                m = half * mh + ml
                ps = ppool.tile([P, T], F32, tag="pbig", name=f"{tag}ps{m}")
                mm(ps, [(wb[k][:, ml * P:(ml + 1) * P], rhs_tiles[k][:])
                        for k in range(nk)])
                epilogue(m, ps)

    def layer_norm(z, gname, pools, mk_out, Tn=T, round_out=True):
        """Feature-dim (partition) LN. z: list of HT [P,Tn] tiles.
        mk_out(k) -> output tile. pools = (pp_stat, lnsq, pbc)."""
        pp_stat, lnsq, pbc = pools
        nchunk = max(1, Tn // NCH)
        cw = min(Tn, NCH)
        ps_s = [pp_stat.tile([1, cw], F32, tag="st", name=f"lnps_s{c}") for c in range(nchunk)]
        ps_q = [pp_stat.tile([1, cw], F32, tag="st", name=f"lnps_q{c}") for c in range(nchunk)]
        for c in range(nchunk):
            for k in range(HT):
                nc.tensor.matmul(ps_s[c][:, :], _fr(ones_col[:, 0:1]),
                                 _fr(z[k][:, c * cw:(c + 1) * cw]),
                                 start=(k == 0), stop=(k == HT - 1))
        for k in range(HT):
            sq = lnsq.tile([P, Tn], F32, tag="lnsq")
            nc.vector.tensor_mul(_fr(sq[:]), z[k][:], z[k][:])
            for c in range(nchunk):
                nc.tensor.matmul(ps_q[c][:, :], _fr(ones_col[:, 0:1]),
                                 _fr(sq[:, c * cw:(c + 1) * cw]),
                                 start=(k == 0), stop=(k == HT - 1))
        rstd = lnsq.tile([1, Tn], F32, tag="rstd", bufs=1, name="rstd")
        mr = lnsq.tile([1, Tn], F32, tag="mr", bufs=1, name="mr")
        for c in range(nchunk):
            cs = slice(c * cw, (c + 1) * cw)
            mean_c = lnsq.tile([1, cw], F32, tag="mean", bufs=2, name="mean_c")
            var_c = lnsq.tile([1, cw], F32, tag="var", bufs=2, name="var_c")
            nc.scalar.activation(mean_c[:], ps_s[c][:], AF.Copy, bias=0.0, scale=1.0 / H)
            nc.vector.tensor_mul(var_c[:], mean_c[:], mean_c[:])
            nc.vector.scalar_tensor_tensor(out=var_c[:], in0=ps_q[c][:], scalar=1.0 / H,
                                           in1=var_c[:], op0=OP.mult, op1=OP.subtract)
            nc.scalar.activation(var_c[:], var_c[:], AF.Sqrt, bias=eps_t[:, 0:1])
            nc.vector.reciprocal(rstd[:, cs], var_c[:])
            nc.vector.tensor_mul(mr[:, cs], mean_c[:], rstd[:, cs])
        bc_r = pbc.tile([P, Tn], F32, tag="bc", name="bc_r")
        bc_mr = pbc.tile([P, Tn], F32, tag="bc", name="bc_mr")
        bcast(bc_r[:], rstd[0:1, 0:Tn], P, Tn, "r")
        bcast(bc_mr[:], mr[0:1, 0:Tn], P, Tn, "mr")
        g_t, b_t = V[f"{gname}_g"], V[f"{gname}_b"]
        outs = []
        cast = _fr if round_out else (lambda a: a)
        for k in range(HT):
            o = mk_out(k)
            nc.vector.tensor_mul(cast(o[:]), z[k][:], bc_r[:])
            nc.vector.tensor_sub(cast(o[:]), o[:], bc_mr[:])
            if not fl[f"{gname}_trivial"]:
                nc.vector.tensor_scalar(out=cast(o[:]), in0=o[:],
                                        scalar1=g_t[:, k:k + 1],
                                        scalar2=b_t[:, k:k + 1], op0=OP.mult, op1=OP.add)
            outs.append(o)
        return outs

    # =========================================================================
    # x^T  (resid generation 1)
    # =========================================================================
    # =========================================================================
    # Stage A: SSM layer
    # =========================================================================
    with tc.tile_pool(name="ssm2", bufs=1) as ssm2, \
         tc.tile_pool(name="ppA", bufs=2, space="PSUM") as ppA:
        states = ssm2.tile([P, T], F32, tag="states")
        wC = ssm2.tile([S, H], FR, tag="wC")
        nc.sync.dma_start(out=wC[:], in_=D["wCT"][:, :])
        with tc.tile_pool(name="ssm1", bufs=1) as ssm1:
            wsg = load_wblocks(ssm1, D["wsgT"], HT, S, "wsg")
            wB = load_wblocks(ssm1, D["wBT"], HT, S, "wB")

            psG = ppA.tile([P, T], F32, tag="pbig")
            mm(psG, [(wsg[k][:], xs[k][:]) for k in range(HT)])
            gate = ssm1.tile([P, T], F32, tag="gate")
            nc.scalar.activation(gate[:], psG[:], AF.Sigmoid, bias=V["sg_b"][:, 0:1])

            psB = ppA.tile([P, T], F32, tag="pbig")
            mm(psB, [(wB[k][:], xs[k][:]) for k in range(HT)])
            u = ssm1.tile([P, T], F32, tag="u")
            nc.vector.tensor_mul(u[:], gate[:], psB[:])

            nc.vector.tensor_tensor_scan(_fr(states[:]),
                                         V["A"][:, 0:1].to_broadcast((P, T)), u[:],
                                         0.0, op0=OP.mult, op1=OP.add)

        with tc.tile_pool(name="lnzA", bufs=8) as lnz, \
             tc.tile_pool(name="lnsqA", bufs=3) as lnsq, \
             tc.tile_pool(name="bcA", bufs=2) as pbc, \
             tc.tile_pool(name="lnoA", bufs=8) as lnout, \
             tc.tile_pool(name="ppstA", bufs=4, space="PSUM") as ppst:
            z1 = []
            for m in range(HT):
                psY = ppA.tile([P, T], F32, tag="pbig", name=f"psY{m}")
                mm(psY, [(wC[:, m * P:(m + 1) * P], states[:])])
                zm = lnz.tile([P, T], F32, tag="z", name=f"z1_{m}")
                nc.vector.scalar_tensor_tensor(out=_fr(zm[:]), in0=xs[m][:],
                                               scalar=V["Dp1"][:, m:m + 1], in1=psY[:],
                                               op0=OP.mult, op1=OP.add)
                z1.append(zm)
            ln1 = layer_norm(z1, "sln", (ppst, lnsq, pbc),
                             lambda k: lnout.tile([P, T], F32, tag="ln1", name=f"ln1_{k}"))

            with tc.tile_pool(name="wouA", bufs=1) as wpo:
                z2 = []

                def ep_outp(m, ps):
                    zm = lnz.tile([P, T], F32, tag="z", name=f"z2_{m}")
                    if fl["outp_b_zero"]:
                        nc.vector.tensor_add(_fr(zm[:]), ps[:], xs[m][:])
                    else:
                        nc.vector.scalar_tensor_tensor(out=_fr(zm[:]), in0=ps[:],
                                                       scalar=V["outp_b"][:, m:m + 1],
                                                       in1=xs[m][:], op0=OP.add, op1=OP.add)
                    z2.append(zm)

                proj("outp_wT", ln1, ep_outp, wpo, ppA, "wou", wbufs=1)
            # x1 = resid generation 2
            x1 = layer_norm(z2, "n1", (ppst, lnsq, pbc),
                            lambda k: rtile(k, f"x1_{k}"))

    # =========================================================================
    # Stage B: self-attention
    # =========================================================================
    Oh = [rtile(g, f"oh{g}") for g in range(HT)]  # resid generation 3
    with tc.tile_pool(name="pQ", bufs=1) as pQ, \
         tc.tile_pool(name="pK", bufs=1) as pK, \
         tc.tile_pool(name="pV", bufs=1) as pV:
        with tc.tile_pool(name="wqkv", bufs=1) as wqkv, \
             tc.tile_pool(name="ppB1", bufs=2, space="PSUM") as ppB1:
            Qh, Kh, Vp = [], [], []

            def ep_q(m, ps):
                qm = pQ.tile([P, T], F32, tag=f"q{m}", name=f"q{m}")
                if fl["a_bq_zero"]:
                    nc.vector.tensor_copy(out=_fr(qm[:]), in_=ps[:])
                else:
                    nc.vector.tensor_scalar_add(_fr(qm[:]), ps[:], V["a_bq"][:, m:m + 1])
                Qh.append(qm)

            def ep_k(m, ps):
                km = pK.tile([P, T], F32, tag=f"k{m}", name=f"k{m}")
                nc.vector.tensor_copy(out=_fr(km[:]), in_=ps[:])
                Kh.append(km)

            proj("a_wqT", x1, ep_q, wqkv, ppB1, "wq", wbufs=1)
            proj("a_wkT", x1, ep_k, wqkv, ppB1, "wq", wbufs=1)
            # V token-major, with a ones column appended per head
            for kt in range(HT):
                vt = pV.tile([P, NH * (DH + 1)], FR, tag=f"v{kt}", name=f"v{kt}")
                nc.sync.dma_start(
                    out=vt[:].rearrange("p (h c) -> p h c", c=DH + 1)[:, :, DH:DH + 1],
                    in_=D["ones16"].rearrange("p (h o) -> p h o", o=1))
                Vp.append(vt)
            for vh in range(2):
                wvb = load_wblocks(wqkv, D["a_wvT"], HT, 512, "wq", c0=vh * 512)
                hs = 512 // (DH + 1) + 1  # 8 heads per half
                for kt in range(HT):
                    ps = ppB1.tile([P, 512], F32, tag="pvh", name=f"psV{vh}_{kt}")
                    mm(ps, [(x1[k][:, kt * P:(kt + 1) * P], wvb[k][:])
                            for k in range(HT)])
                    nc.vector.tensor_copy(
                        out=Vp[kt][:].rearrange("p (h c) -> p h c", c=DH + 1)[
                            :, 8 * vh:8 * (vh + 1), 0:DH],
                        in_=ps[:].rearrange("p (h c) -> p h c", c=DH)[:, :, :])

        with tc.tile_pool(name="pexp", bufs=4) as pexp, \
             tc.tile_pool(name="pbcB", bufs=2) as pbcB, \
             tc.tile_pool(name="ppSC", bufs=2, space="PSUM") as ppSC, \
             tc.tile_pool(name="ppAV", bufs=2, space="PSUM") as ppAV:
            for h in range(NH):
                g, ho = h // 2, (h % 2) * DH
                psA = ppAV.tile([DH + 1, T], F32, tag="pav", name=f"pav{h}")
                for kt in range(HT):
                    psS = ppSC.tile([P, T], F32, tag="psc", name=f"psc{h}_{kt}")
                    mm(psS, [(Kh[g][ho:ho + DH, kt * P:(kt + 1) * P],
                              Qh[g][ho:ho + DH, :])])
                    et = pexp.tile([P, T], FR, tag="exp", name=f"exp{h}_{kt}")
                    nc.scalar.activation(et[:], psS[:], AF.Exp, scale=1.0 / 8.0)
                    vslice = Vp[kt][:, h * (DH + 1):(h + 1) * (DH + 1)]
                    for c0 in range(0, T, NCH):
                        nc.tensor.matmul(psA[:, c0:c0 + NCH], _fr(vslice),
                                         _fr(et[:, c0:c0 + NCH]),
                                         start=(kt == 0), stop=(kt == HT - 1))
                rcp = sm.tile([1, T], F32, tag="stat", name=f"rcp{h}")
                nc.vector.reciprocal(rcp[:], psA[DH:DH + 1, :])
                rb = pbcB.tile([DH, T], F32, tag="rb", name=f"rb{h}")
                bcast(rb[:], rcp[0:1, :], DH, T, "rc")
                nc.vector.tensor_mul(_fr(Oh[g][ho:ho + DH, :]), psA[0:DH, :], rb[:])

    with tc.tile_pool(name="lnzB", bufs=8) as lnz, \
         tc.tile_pool(name="lnsqB", bufs=3) as lnsq, \
         tc.tile_pool(name="bcB2", bufs=2) as pbc, \
         tc.tile_pool(name="woB", bufs=1) as wpo, \
         tc.tile_pool(name="ppB3", bufs=2, space="PSUM") as ppB3, \
         tc.tile_pool(name="ppstB", bufs=4, space="PSUM") as ppst:
        z3 = []

        def ep_wo(m, ps):
            zm = lnz.tile([P, T], F32, tag="z", name=f"z3_{m}")
            if fl["a_const_zero"]:
                nc.vector.tensor_add(_fr(zm[:]), ps[:], x1[m][:])
            else:
                nc.vector.scalar_tensor_tensor(out=_fr(zm[:]), in0=ps[:],
                                               scalar=V["a_const"][:, m:m + 1],
                                               in1=x1[m][:], op0=OP.add, op1=OP.add)
            z3.append(zm)

        proj("a_woT", Oh, ep_wo, wpo, ppB3, "wo")
        # x2 = resid generation 4
        x2 = layer_norm(z3, "n2", (ppst, lnsq, pbc),
                        lambda k: rtile(k, f"x2_{k}"))

    # =========================================================================
    # Stage C: hierarchical memory retrieval + gated merge
    # =========================================================================
    cstk = ExitStack()
    with cstk:
        pKr = cstk.enter_context(tc.tile_pool(name="pKr", bufs=1))
        pVr = cstk.enter_context(tc.tile_pool(name="pVr", bufs=1))
        Kr, Vr = {}, {}
        with tc.tile_pool(name="pc", bufs=1) as pc:
            chat = {}
            with tc.tile_pool(name="cw", bufs=1) as cw, \
                 tc.tile_pool(name="cmid", bufs=1) as cmid, \
                 tc.tile_pool(name="ppC1", bufs=6, space="PSUM") as ppC1:
                for i, cwid in enumerate(COMP):
                    ct = cwid // P
                    with tc.tile_pool(name=f"pmT{i}", bufs=1) as pmT:
                        mT = load_wblocks(pmT, D[f"m{i}T"], HT, 256, "mT")
                        w1 = load_wblocks(cw, D[f"c{i}_w1T"], HT, cwid, "cwx")
                        mid = []
                        for cm in range(ct):
                            ps = ppC1.tile([P, 256], F32, tag="pc1", name=f"pm{i}_{cm}")
                            mm(ps, [(w1[k][:, cm * P:(cm + 1) * P], mT[k][:])
                                    for k in range(HT)])
                            md = cmid.tile([P, 256], FR, tag=f"mid{cm}", name=f"mid{i}_{cm}")
                            nc.scalar.activation(md[:], ps[:], AF.Relu,
                                                 bias=V[f"c{i}_b1"][:, cm:cm + 1])
                            mid.append(md)
                        w2 = load_wblocks(cw, D[f"c{i}_w2T"], ct, H, "cwx")
                        for m in range(HT):
                            ps = ppC1.tile([P, 256], F32, tag="pc1", name=f"pc{i}_{m}")
                            mm(ps, [(w2[k][:, m * P:(m + 1) * P], mid[k][:])
                                    for k in range(ct)])
                            cm_t = pc.tile([P, 256], F32, tag=f"c{i}_{m}", name=f"c{i}_{m}")
                            if fl[f"c{i}_b2_zero"]:
                                nc.vector.tensor_copy(out=_fr(cm_t[:]), in_=ps[:])
                            else:
                                nc.vector.tensor_scalar_add(_fr(cm_t[:]), ps[:],
                                                            V[f"c{i}_b2"][:, m:m + 1])
                            chat.setdefault(i, []).append(cm_t)
            with tc.tile_pool(name="rkv", bufs=1) as rkv, \
                 tc.tile_pool(name="ppC2", bufs=2, space="PSUM") as ppC2:
                wkr = load_wblocks(rkv, D["r_wkT"], HT, H, "rkv")
                for i in range(3):
                    Kr[i] = []
                    for m in range(HT):
                        ps = ppC2.tile([P, 256], F32, tag="pkv", name=f"pk{i}_{m}")
                        mm(ps, [(wkr[k][:, m * P:(m + 1) * P], chat[i][k][:])
                                for k in range(HT)])
                        kt_ = pKr.tile([P, 256], F32, tag=f"kr{i}_{m}", name=f"kr{i}_{m}")
                        nc.vector.tensor_copy(out=_fr(kt_[:]), in_=ps[:])
                        Kr[i].append(kt_)
                wvr = load_wblocks(rkv, D["r_wvT"], HT, H, "rkv")
                for i in range(3):
                    Vr[i] = []
                    for kvt in range(2):
                        ps = ppC2.tile([P, T], F32, tag="pkv2", name=f"pv{i}_{kvt}")
                        mm(ps, [(chat[i][k][:, kvt * P:(kvt + 1) * P], wvr[k][:])
                                for k in range(HT)])
                        vt = pVr.tile([P, T], F32, tag=f"vr{i}_{kvt}", name=f"vr{i}_{kvt}")
                        nc.vector.tensor_copy(out=_fr(vt[:]), in_=ps[:])
                        Vr[i].append(vt)
        pQr = cstk.enter_context(tc.tile_pool(name="pQr", bufs=1))
        with tc.tile_pool(name="rwq", bufs=1) as rwq, \
             tc.tile_pool(name="ppC3", bufs=3, space="PSUM") as ppC3:
            Qr = []

            def ep_qr(m, ps):
                qm = pQr.tile([P, T], F32, tag=f"qr{m}", name=f"qr{m}")
                if fl["r_bq_zero"]:
                    nc.vector.tensor_copy(out=_fr(qm[:]), in_=ps[:])
                else:
                    nc.vector.tensor_scalar_add(_fr(qm[:]), ps[:], V["r_bq"][:, m:m + 1])
                Qr.append(qm)

            proj("r_wqT", x2, ep_qr, rwq, ppC3, "rwq")

        Or = [rtile(m, f"orr{m}") for m in range(HT)]  # resid generation 5
        with tc.tile_pool(name="pexpR", bufs=4) as pexpR, \
             tc.tile_pool(name="ptwR", bufs=2) as ptw, \
             tc.tile_pool(name="pbcR", bufs=3) as pbcR, \
             tc.tile_pool(name="ppSCr", bufs=1, space="PSUM") as ppSCr, \
             tc.tile_pool(name="ppAVr", bufs=2, space="PSUM") as ppAVr, \
             tc.tile_pool(name="ppsum", bufs=2, space="PSUM") as ppsum:
            for i in range(3):
                for r in range(RH):
                    ets = []
                    for kvt in range(2):
                        psS = ppSCr.tile([P, T], F32, tag="psc", name=f"rsc{i}{r}{kvt}")
                        mm(psS, [(Kr[i][2 * r + kc][:, kvt * P:(kvt + 1) * P],
                                  Qr[2 * r + kc][:]) for kc in range(2)])
                        et = pexpR.tile([P, T], FR, tag="expr", name=f"re{i}{r}{kvt}")
                        nc.scalar.activation(et[:], psS[:], AF.Exp, scale=1.0 / 16.0)
                        ets.append(et)
                    rcp = sm.tile([1, T], F32, tag="stat", name=f"rcpr{i}{r}")
                    for c0 in range(0, T, NCH):
                        psZ = ppsum.tile([1, NCH], F32, tag="pz", name=f"rz{i}{r}{c0}")
                        for kvt in range(2):
                            nc.tensor.matmul(psZ[:, :], _fr(ones_col[:, 0:1]),
                                             _fr(ets[kvt][:, c0:c0 + NCH]),
                                             start=(kvt == 0), stop=(kvt == 1))
                        nc.vector.reciprocal(rcp[:, c0:c0 + NCH], psZ[:, :])
                    rb = pbcR.tile([P, T], F32, tag="rbr", name=f"rbr{i}{r}")
                    bcast(rb[:], rcp[0:1, :], P, T, "rr")
                    for md in range(2):
                        psA = ppAVr.tile([P, T], F32, tag="pav", name=f"rav{i}{r}{md}")
                        col = RDH * r + P * md
                        for c0 in range(0, T, NCH):
                            for kvt in range(2):
                                nc.tensor.matmul(psA[:, c0:c0 + NCH],
                                                 _fr(Vr[i][kvt][:, col:col + P]),
                                                 _fr(ets[kvt][:, c0:c0 + NCH]),
                                                 start=(kvt == 0), stop=(kvt == 1))
                        dst = Or[2 * r + md]
                        if i == 0:
                            nc.vector.tensor_mul(_fr(dst[:]), psA[:], rb[:])
                        else:
                            tw = ptw.tile([P, T], F32, tag="tw", name=f"tw{i}{r}{md}")
                            nc.vector.tensor_mul(tw[:], psA[:], rb[:])
                            nc.vector.tensor_add(_fr(dst[:]), dst[:], tw[:])

    with tc.tile_pool(name="pcomb", bufs=1) as pcomb:
        with tc.tile_pool(name="rwo", bufs=1) as rwo, \
             tc.tile_pool(name="ppC5", bufs=3, space="PSUM") as ppC5:
            comb = []

            def ep_ro(m, ps):
                cm_ = pcomb.tile([P, T], F32, tag=f"cb{m}", name=f"cb{m}")
                if fl["r_const_zero"]:
                    nc.scalar.activation(_fr(cm_[:]), ps[:], AF.Copy, bias=0.0, scale=1.0 / 3.0)
                else:
                    nc.vector.tensor_scalar(out=_fr(cm_[:]), in0=ps[:], scalar1=1.0 / 3.0,
                                            scalar2=V["r_const"][:, m:m + 1],
                                            op0=OP.mult, op1=OP.add)
                comb.append(cm_)

            proj("r_woT", Or, ep_ro, rwo, ppC5, "rwo")

        x3 = []
        with tc.tile_pool(name="mgw", bufs=1) as mgw, \
             tc.tile_pool(name="pgw", bufs=2) as pgw, \
             tc.tile_pool(name="ptmp", bufs=2) as ptmp, \
             tc.tile_pool(name="ppC6", bufs=3, space="PSUM") as ppC6:
            for half in range(2):
                wb = load_wblocks(mgw, D["mg_wT"], 2 * HT, 4 * P, "mg", c0=half * 4 * P,
                                  bufs=2)
                for ml in range(4):
                    m = half * 4 + ml
                    ps = ppC6.tile([P, T], F32, tag="pbig", name=f"mgps{m}")
                    steps = [(wb[k][:, ml * P:(ml + 1) * P], x2[k][:]) for k in range(HT)]
                    steps += [(wb[HT + k][:, ml * P:(ml + 1) * P], comb[k][:])
                              for k in range(HT)]
                    mm(ps, steps)
                    gw = pgw.tile([P, T], F32, tag="gw", name=f"gw{m}")
                    nc.scalar.activation(gw[:], ps[:], AF.Sigmoid, bias=V["mg_b"][:, m:m + 1])
                    d = ptmp.tile([P, T], F32, tag="d", name=f"d{m}")
                    nc.vector.tensor_sub(d[:], x2[m][:], comb[m][:])
                    nc.vector.tensor_mul(d[:], gw[:], d[:])
                    s = ptmp.tile([P, T], F32, tag="s", name=f"s{m}")
                    nc.vector.tensor_add(s[:], x2[m][:], comb[m][:])
                    xm = rtile(m, f"x3_{m}")  # resid generation 6
                    nc.vector.tensor_add(_fr(xm[:]), s[:], d[:])
                    x3.append(xm)

    # =========================================================================
    # Stage D: FFN in token-halves + final LN + transpose to [T, H]
    # =========================================================================
    TH = T // 2
    with tc.tile_pool(name="ph", bufs=1) as ph:
        hts = {0: [], 1: []}
        with tc.tile_pool(name="fw1", bufs=2) as fw1, \
             tc.tile_pool(name="ppD1", bufs=4, space="PSUM") as ppD1:
            for mg_i in range(8):
                wblk = load_wblocks(fw1, D["f_w1T"], HT, 512, "w1s",
                                    c0=mg_i * 512, bufs=2)
                for th in range(2):
                    c0 = th * TH
                    for ml in range(4):
                        m_abs = mg_i * 4 + ml
                        ps = ppD1.tile([P, TH], F32, tag="p1", name=f"f1ps{th}_{m_abs}")
                        mm(ps, [(wblk[k][:, ml * P:(ml + 1) * P],
                                 x3[k][:, c0:c0 + TH]) for k in range(HT)])
                        htile = ph.tile([P, TH], BF16, tag=f"h{th}_{m_abs}",
                                        name=f"h{th}_{m_abs}")
                        nc.scalar.activation(htile[:], ps[:], AF.Gelu,
                                             bias=V["f_b1"][:, m_abs:m_abs + 1])
                        hts[th].append(htile)
        for th in range(2):
            c0 = th * TH
            with tc.tile_pool(name="lnzD", bufs=8) as lnz:
                z4 = []
                with tc.tile_pool(name="fw2", bufs=4) as fw2, \
                     tc.tile_pool(name="ppD2", bufs=1, space="PSUM") as ppD2:
                    pso = [ppD2.tile([P, TH], F32, tag=f"p2_{m}", name=f"pso{th}_{m}")
                           for m in range(HT)]
                    for k2 in range(4 * HT):
                        wt = fw2.tile([P, H], BF16, tag="w2s", name=f"w2s{th}_{k2}")
                        nc.sync.dma_start(out=wt[:], in_=D["f_w2T"][k2 * P:(k2 + 1) * P, :])
                        for mo in range(HT):
                            nc.tensor.matmul(pso[mo][:, :],
                                             wt[:, mo * P:(mo + 1) * P],
                                             hts[th][k2][:],
                                             start=(k2 == 0), stop=(k2 == 4 * HT - 1))
                    for mo in range(HT):
                        zm = lnz.tile([P, TH], F32, tag="z", name=f"z4_{th}_{mo}")
                        if fl["f_b2_zero"]:
                            nc.vector.tensor_add(_fr(zm[:]), pso[mo][:], x3[mo][:, c0:c0 + TH])
                        else:
                            nc.vector.scalar_tensor_tensor(out=_fr(zm[:]), in0=pso[mo][:],
                                                           scalar=V["f_b2"][:, mo:mo + 1],
                                                           in1=x3[mo][:, c0:c0 + TH],
                                                           op0=OP.add, op1=OP.add)
                        z4.append(zm)
                with tc.tile_pool(name="lnsqD", bufs=2) as lnsq, \
                     tc.tile_pool(name="bcD", bufs=2) as pbc, \
                     tc.tile_pool(name="lnoD", bufs=8) as lnout, \
                     tc.tile_pool(name="stg", bufs=2) as stg_pool, \
                     tc.tile_pool(name="ppstD", bufs=4, space="PSUM") as ppst, \
                     tc.tile_pool(name="ppT", bufs=4, space="PSUM") as ppT:
                    fin = layer_norm(z4, "n3", (ppst, lnsq, pbc),
                                     lambda k: lnout.tile([P, TH], F32, tag="fin",
                                                          name=f"fin{th}_{k}"),
                                     Tn=TH, round_out=False)
                    for tt in range(TH // P):
                        stg = stg_pool.tile([P, H], F32, tag="stg", name=f"stg{th}_{tt}")
                        for k2 in range(HT):
                            psT = ppT.tile([P, P], F32, tag="pt", name=f"pT{th}_{tt}_{k2}")
                            nc.tensor.transpose(psT[:, :],
                                                fin[k2][:, tt * P:(tt + 1) * P],
                                                ident[:])
                            nc.vector.tensor_copy(out=stg[:, k2 * P:(k2 + 1) * P],
                                                  in_=psT[:, :])
                        row0 = c0 + tt * P
                        nc.sync.dma_start(out=out_d[row0:row0 + P, :], in_=stg[:])
    ctx.close()


# =============================================================================
# Host side
# =============================================================================
_CACHE = {}


def _flags(g):
    def zero(a):
        return bool(np.all(a == 0.0))

    fl = {}
    for n in ("sln", "n1", "n2", "n3"):
        fl[f"{n}_trivial"] = bool(np.all(g[f"{n}_g"] == 1.0) and zero(g[f"{n}_b"]))
    fl["outp_b_zero"] = zero(g["outp_b"])
    wq_b, wk_b, wv_b = np.split(g["attn_in_b"], 3, 0)
    fl["a_bq_zero"] = zero(wq_b)
    a_const = wv_b @ g["attn_out_w"].T + g["attn_out_b"]
    fl["a_const_zero"] = zero(a_const)
    rq_b, rk_b, rv_b = np.split(g["retr_in_b"], 3, 0)
    fl["r_bq_zero"] = zero(rq_b)
    r_const = rv_b @ g["retr_out_w"].T + g["retr_out_b"]
    fl["r_const_zero"] = zero(r_const)
    for i in range(3):
        fl[f"c{i}_b2_zero"] = zero(g[f"c{i}_b2"])
    fl["f_b2_zero"] = zero(g["ffn_b2"])
    return fl, a_const, r_const


def kernel(**inputs):
    g = {k: np.ascontiguousarray(np.asarray(v, dtype=np.float32)) for k, v in inputs.items()}
    fl, a_const, r_const = _flags(g)

    key = tuple(sorted(fl.items()))
    if key not in _CACHE:
        _CACHE[key] = build_nc(fl)
    nc = _CACHE[key]

    def tr(a):
        return np.ascontiguousarray(a.T)

    wq, wk, wv = np.split(g["attn_in_w"], 3, 0)
    rq, rk, rv = np.split(g["retr_in_w"], 3, 0)
    shared = {
        "wsgT": tr(g["sgate_w"]), "wBT": tr(g["B_w"]), "wCT": tr(g["C_w"]),
        "A": np.exp(g["A_log"]).reshape(S, 1), "sg_b": g["sgate_b"].reshape(S, 1),
        "Dp1": (g["D"] + 1.0).reshape(H, 1),
        "outp_wT": tr(g["outp_w"]), "outp_b": g["outp_b"].reshape(H, 1),
        "a_wqT": tr(wq), "a_wkT": tr(wk), "a_wvT": tr(wv),
        "a_bq": np.split(g["attn_in_b"], 3, 0)[0].reshape(H, 1),
        "a_woT": tr(g["attn_out_w"]), "a_const": a_const.reshape(H, 1),
        "r_wqT": tr(rq), "r_wkT": tr(rk), "r_wvT": tr(rv),
        "r_bq": np.split(g["retr_in_b"], 3, 0)[0].reshape(H, 1),
        "r_woT": tr(g["retr_out_w"]), "r_const": r_const.reshape(H, 1),
        "mg_wT": tr(g["mg_w"]), "mg_b": g["mg_b"].reshape(H, 1),
        "f_w1T": tr(g["ffn_w1"]), "f_b1": g["ffn_b1"].reshape(4 * H, 1),
        "f_b2": g["ffn_b2"].reshape(H, 1),
    }
    for i in range(3):
        shared[f"c{i}_w1T"] = tr(g[f"c{i}_w1"])
        shared[f"c{i}_b1"] = g[f"c{i}_b1"].reshape(COMP[i], 1)
        shared[f"c{i}_w2T"] = tr(g[f"c{i}_w2"])
        shared[f"c{i}_b2"] = g[f"c{i}_b2"].reshape(H, 1)
    for n in ("sln", "n1", "n2", "n3"):
        shared[f"{n}_g"] = g[f"{n}_g"].reshape(H, 1)
        shared[f"{n}_b"] = g[f"{n}_b"].reshape(H, 1)
    shared["ones"] = np.ones((P, 1), np.float32)
    shared["ones16"] = np.ones((P, NH), np.float32)
    shared = {k: np.ascontiguousarray(v.astype(np.float32)) for k, v in shared.items()}
    import ml_dtypes
    shared["f_w2T"] = np.ascontiguousarray(tr(g["ffn_w2"]).astype(ml_dtypes.bfloat16))

    in_maps = []
    for b in range(B):
        m = dict(shared)
        m["xT"] = tr(g["x"][b])
        for i in range(3):
            m[f"m{i}T"] = tr(g[f"mem{i}"][b, -256:, :])
        in_maps.append(m)

    trace = os.environ.get("KERNEL_TRACE", "0") == "1"
    res = bass_utils.run_bass_kernel_spmd(nc, in_maps, core_ids=list(range(B)),
                                          trace=trace)
    global LAST_RESULTS
    LAST_RESULTS = res
    out = np.stack([res.results[b]["out"] for b in range(B)], axis=0)
    return out


LAST_RESULTS = None


def bench(n_iter=6, **inputs):
    """Time the on-device execution with device-resident inputs (excludes
    host->device transfer). Returns (best_seconds, out)."""
    import time

    import jax
    import jax.numpy as jnp
    from jax.sharding import Mesh, PartitionSpec
    from jax.experimental.shard_map import shard_map
    from concourse import bass2jax

    g = {k: np.ascontiguousarray(np.asarray(v, dtype=np.float32)) for k, v in inputs.items()}
    fl, a_const, r_const = _flags(g)
    key = tuple(sorted(fl.items()))
    if key not in _CACHE:
        _CACHE[key] = build_nc(fl)
    nc = _CACHE[key]
    in_maps = _in_maps(g, a_const, r_const)

    bass2jax.install_neuronx_cc_hook()
    import concourse.mybir as mybir_
    in_names, out_names, out_avals, zero_outs = [], [], [], []
    for alloc in nc.m.functions[0].allocations:
        if not isinstance(alloc, mybir_.MemoryLocationSet):
            continue
        name = alloc.memorylocations[0].name
        pid_name = nc.partition_id_tensor.name if nc.partition_id_tensor else None
        if alloc.kind == "ExternalInput":
            if name != pid_name:
                in_names.append(name)
        elif alloc.kind == "ExternalOutput":
            out_names.append(name)
            np_dt = mybir_.dt.np(alloc.dtype)
            out_avals.append(jax.core.ShapedArray(tuple(alloc.tensor_shape), np_dt))
            zero_outs.append(np.zeros(tuple(alloc.tensor_shape), np_dt))
    n_params = len(in_names)
    all_names = in_names + out_names
    if nc.partition_id_tensor is not None:
        all_names = all_names + [nc.partition_id_tensor.name]

    def _body(*args):
        operands = list(args)
        if nc.partition_id_tensor is not None:
            operands.append(bass2jax.partition_id_tensor())
        outs = bass2jax._bass_exec_p.bind(
            *operands, out_avals=tuple(out_avals), in_names=tuple(all_names),
            out_names=tuple(out_names), lowering_input_output_aliases=(),
            sim_require_finite=True, sim_require_nnan=True, nc=nc)
        return tuple(outs)

    devices = jax.devices()[:B]
    mesh = Mesh(np.asarray(devices), ("core",))
    nin = n_params + len(out_names)
    sh = jax.sharding.NamedSharding(mesh, PartitionSpec("core"))
    concat_in = [np.concatenate([np.asarray(in_maps[c][i_name])
                                 for c in range(B)], axis=0) for i_name in in_names]
    concat_zeros = [np.zeros((B * z.shape[0], *z.shape[1:]), z.dtype) for z in zero_outs]
    dev_in = [jax.device_put(a, sh) for a in concat_in + concat_zeros]
    jax.block_until_ready(dev_in)

    def _compile():
        return jax.jit(shard_map(
            _body, mesh=mesh,
            in_specs=(PartitionSpec("core"),) * nin,
            out_specs=(PartitionSpec("core"),) * len(out_names),
            check_rep=False)).lower(*dev_in).compile()

    try:
        sharded = bass2jax.fast_dispatch_compile(_compile)
    except Exception as e:
        print(f"  (fast dispatch unavailable: {e})")
        sharded = jax.jit(shard_map(_body, mesh=mesh,
                                    in_specs=(PartitionSpec("core"),) * nin,
                                    out_specs=(PartitionSpec("core"),) * len(out_names),
                                    check_rep=False))
    best = None
    out = None
    for it in range(n_iter):
        t0 = time.perf_counter()
        out = sharded(*dev_in)
        jax.block_until_ready(out)
        dt = time.perf_counter() - t0
        print(f"  iter {it}: {dt * 1e3:.2f} ms")
        if best is None or dt < best:
            best = dt
    # pure-jax dispatch floor on the same mesh
    tiny = jax.jit(lambda a: a + 1.0)
    ta = jax.device_put(np.zeros((8, 128), np.float32),
                        jax.sharding.NamedSharding(mesh, PartitionSpec("core")))
    jax.block_until_ready(tiny(ta))
    t0 = time.perf_counter()
    for _ in range(8):
        jax.block_until_ready(tiny(ta))
    print(f"  tiny-op sync floor: {(time.perf_counter() - t0) / 8 * 1e3:.2f} ms")
    t0 = time.perf_counter()
    outs = [tiny(ta) for _ in range(16)]
    jax.block_until_ready(outs)
    print(f"  tiny-op queued floor: {(time.perf_counter() - t0) / 16 * 1e3:.2f} ms")
    # amortized: pipeline several calls to hide RPC latency; repeat rounds
    # and keep the best to shed tunnel-latency noise
    nq = 16
    amort = None
    for rnd in range(4):
        t0 = time.perf_counter()
        outs = [sharded(*dev_in) for _ in range(nq)]
        jax.block_until_ready(outs)
        dt = (time.perf_counter() - t0) / nq
        print(f"  amortized over {nq} queued calls (round {rnd}): {dt * 1e3:.2f} ms")
        if amort is None or dt < amort:
            amort = dt
    res = np.asarray(out[0]).reshape(B, T, H)
    return min(best, amort), res


def _in_maps(g, a_const, r_const):
    def tr(a):
        return np.ascontiguousarray(a.T)

    wq, wk, wv = np.split(g["attn_in_w"], 3, 0)
    rq, rk, rv = np.split(g["retr_in_w"], 3, 0)
    shared = {
        "wsgT": tr(g["sgate_w"]), "wBT": tr(g["B_w"]), "wCT": tr(g["C_w"]),
        "A": np.exp(g["A_log"]).reshape(S, 1), "sg_b": g["sgate_b"].reshape(S, 1),
        "Dp1": (g["D"] + 1.0).reshape(H, 1),
        "outp_wT": tr(g["outp_w"]), "outp_b": g["outp_b"].reshape(H, 1),
        "a_wqT": tr(wq), "a_wkT": tr(wk), "a_wvT": tr(wv),
        "a_bq": np.split(g["attn_in_b"], 3, 0)[0].reshape(H, 1),
        "a_woT": tr(g["attn_out_w"]), "a_const": a_const.reshape(H, 1),
        "r_wqT": tr(rq), "r_wkT": tr(rk), "r_wvT": tr(rv),
        "r_bq": np.split(g["retr_in_b"], 3, 0)[0].reshape(H, 1),
        "r_woT": tr(g["retr_out_w"]), "r_const": r_const.reshape(H, 1),
        "mg_wT": tr(g["mg_w"]), "mg_b": g["mg_b"].reshape(H, 1),
        "f_w1T": tr(g["ffn_w1"]), "f_b1": g["ffn_b1"].reshape(4 * H, 1),
        "f_b2": g["ffn_b2"].reshape(H, 1),
    }
    for i in range(3):
        shared[f"c{i}_w1T"] = tr(g[f"c{i}_w1"])
        shared[f"c{i}_b1"] = g[f"c{i}_b1"].reshape(COMP[i], 1)
        shared[f"c{i}_w2T"] = tr(g[f"c{i}_w2"])
        shared[f"c{i}_b2"] = g[f"c{i}_b2"].reshape(H, 1)
    for n in ("sln", "n1", "n2", "n3"):
        shared[f"{n}_g"] = g[f"{n}_g"].reshape(H, 1)
        shared[f"{n}_b"] = g[f"{n}_b"].reshape(H, 1)
    shared["ones"] = np.ones((P, 1), np.float32)
    shared["ones16"] = np.ones((P, NH), np.float32)
    shared = {k: np.ascontiguousarray(v.astype(np.float32)) for k, v in shared.items()}
    import ml_dtypes
    shared["f_w2T"] = np.ascontiguousarray(tr(g["ffn_w2"]).astype(ml_dtypes.bfloat16))
    in_maps = []
    for b in range(B):
        m = dict(shared)
        m["xT"] = tr(g["x"][b])
        for i in range(3):
            m[f"m{i}T"] = tr(g[f"mem{i}"][b, -256:, :])
        in_maps.append(m)
    return in_maps



# revision 8
# speedup vs baseline: 5.0751x; 5.0070x over previous
"""TRN2 Bass/Tile kernel for nn_DHSMBlock (SSM + self-attn + hierarchical memory + FFN).

Sharding: data-parallel over batch. B=8 rows -> 8 NeuronCores, one row per core,
no collectives. Each core gets the full weight set (host pre-transposed).

On-device layout is feature-major: every activation lives as X^T [feature, token]
so that all matmuls contract over the partition dim. Weights are shipped as W^T
[in_f, out_f] (host numpy transpose). LayerNorm is over the feature dim =
partition dim; stats are computed with ones-vector matmuls on the PE and
broadcast back with SBUF->SBUF stride-0 DMAs. The SSM recurrence is a single
DVE tensor_tensor_scan instruction per core. Softmax is computed k-major
(scores^T), so no transposes are needed anywhere except the final output.
"""

import os
from contextlib import ExitStack

import numpy as np

os.environ.setdefault("MYCRO_LOCAL_CACHE", "1")

import concourse.bass as bass
import concourse.mybir as mybir
import concourse.tile as tile
from concourse import bass_utils
from concourse.masks import make_identity

F32 = mybir.dt.float32
FR = mybir.dt.float32r
BF16 = mybir.dt.bfloat16
AF = mybir.ActivationFunctionType
OP = mybir.AluOpType

B, T, H, S = 8, 1024, 1024, 128
NH, DH = 16, 64          # self-attention heads
RH, RDH = 4, 256         # retriever heads
COMP = [1024, 512, 256]  # compressor widths
P = 128
HT = H // P              # 8 feature tiles
NCH = 512                # matmul moving-dim chunk (one fp32 PSUM bank)
EPS = 1e-5


def _fr(ap):
    return ap.bitcast(FR)


def build_nc(fl, loop_n=1):
    """Build the Bass program. fl: dict of host-known triviality flags.

    loop_n > 1 wraps the whole kernel body in a hardware loop (tc.For_i), so
    one NEFF execution performs loop_n identical full computations — used by
    bench() to measure per-execution device time net of dispatch overhead.
    """
    nc = bass.Bass("TRN2", target_bir_lowering=False, debug=False, num_devices=8)
    D = {}

    def din(name, shape, dt=F32):
        D[name] = nc.dram_tensor(name, list(shape), dt, kind="ExternalInput").ap()

    din("xT", (H, T), FR)
    for i in range(3):
        din(f"m{i}T", (H, 256), FR)
    din("wsgT", (H, S), FR); din("wBT", (H, S), FR); din("wCT", (S, H), FR)
    din("A", (S, 1)); din("sg_b", (S, 1)); din("Dp1", (H, 1))
    din("outp_wT", (H, H), FR); din("outp_b", (H, 1))
    din("a_wqT", (H, H), FR); din("a_wkT", (H, H), FR); din("a_wvT", (H, H), FR)
    din("a_bq", (H, 1)); din("a_woT", (H, H), FR); din("a_const", (H, 1))
    for i, c in enumerate(COMP):
        din(f"c{i}_w1T", (H, c), FR); din(f"c{i}_b1", (c, 1))
        din(f"c{i}_w2T", (c, H), FR); din(f"c{i}_b2", (H, 1))
    din("r_wqT", (H, H), FR); din("r_wkT", (H, H), FR); din("r_wvT", (H, H), FR)
    din("r_bq", (H, 1)); din("r_woT", (H, H), FR); din("r_const", (H, 1))
    din("mg_wT", (2 * H, H), FR); din("mg_b", (H, 1))
    din("f_w1T", (H, 4 * H), FR); din("f_b1", (4 * H, 1))
    din("f_w2T", (4 * H, H), BF16); din("f_b2", (H, 1))
    for n in ("sln", "n1", "n2", "n3"):
        din(f"{n}_g", (H, 1)); din(f"{n}_b", (H, 1))
    din("ones", (P, 1), FR)
    din("ones16", (P, NH), FR)
    out_d = nc.dram_tensor("out", [T, H], F32, kind="ExternalOutput").ap()

    with tile.TileContext(nc, pool_alloc_mode="queue") as tc:
        if loop_n > 1:
            with tc.For_i(0, loop_n):
                _body(nc, tc, D, out_d, fl)
        else:
            _body(nc, tc, D, out_d, fl)
    _split_matmul_waits(nc)
    return nc


_WAIT_EXEMPT = {
    "InstEventSemaphore", "InstAllEngineBarrier",
    "InstUnconditionalBranch", "InstCompareAndBranch", "InstIndirectBranch",
    "InstHalt", "InstBranchHint",
}


def _split_matmul_waits(nc):
    """TPB engine instruction encodings carry at most one sync wait; move
    surplus waits onto a preceding same-engine no-op (sequencer WAITs)."""
    import bass_rust
    cnt = 0
    for f in nc.m.functions:
        for blk in f.blocks:
            insts = blk.instructions
            out = []
            changed = False
            for inst in insts:
                if (type(inst).__name__ not in _WAIT_EXEMPT
                        and not isinstance(inst, bass_rust.InstISA)):
                    si = inst.sync_info
                    if si is not None and len(si.on_wait) > 1:
                        surplus = list(si.on_wait[:-1])
                        # each EventSemaphore carries at most 2 waits
                        for j in range(0, len(surplus), 2):
                            ev = bass_rust.InstEventSemaphore(name=f"I-wsplit-{cnt}")
                            cnt += 1
                            ev.engine = inst.engine
                            ev.bass_nofuse = True
                            ev.sync_info = bass_rust.SyncInfo(
                                on_wait=surplus[j:j + 2], on_update=[])
                            out.append(ev)
                        inst.sync_info = bass_rust.SyncInfo(
                            on_wait=[si.on_wait[-1]], on_update=list(si.on_update))
                        changed = True
                out.append(inst)
            if changed:
                blk.instructions = out


def _body(nc, tc, D, out_d, fl):
    import itertools
    _bc_ctr = itertools.count()
    ctx = ExitStack()

    # ---------- ambient pools ----------
    pv = ctx.enter_context(tc.tile_pool(name="pv", bufs=1))
    sm = ctx.enter_context(tc.tile_pool(name="sm", bufs=2))
    # Residual-chain ring: tags r0..r7, two generations in flight per tag.
    # Generation order per tag: x -> x1 -> O -> x2 -> Or -> x3.
    resid = ctx.enter_context(tc.tile_pool(name="resid", bufs=2))
    dscr = ctx.enter_context(tc.tile_pool(name="dscr", bufs=4, space="DRAM"))

    def bcast(dst_ap, src_ap, parts, tn, tag):
        """Broadcast a [1,tn] SBUF row to [parts,tn] via a DRAM round-trip
        (engines cannot read partition-stride-0 SBUF APs; DRAM DMAs can)."""
        scr = dscr.tile([1, tn], F32, tag=tag, name=f"scr_{tag}_{next(_bc_ctr)}")
        nc.sync.dma_start(out=scr[:], in_=src_ap)
        nc.sync.dma_start(out=dst_ap, in_=scr[0:1, :].broadcast_to((parts, tn)))

    def rtile(k, name):
        return resid.tile([P, T], F32, tag=f"r{k}", name=name)

    def vec_tile(name, rows):
        nt = rows // P
        t = pv.tile([P, nt], F32, tag=name, name=f"v_{name}")
        nc.sync.dma_start(out=t[:], in_=D[name].rearrange("(k p) o -> p (k o)", p=P))
        return t

    xs = []
    for k in range(HT):
        t = rtile(k, f"x_{k}")
        nc.sync.dma_start(out=_fr(t[:]), in_=D["xT"][k * P:(k + 1) * P, :])
        xs.append(t)

    V = {}
    for name, rows in [
        ("sg_b", S), ("A", S), ("Dp1", H), ("outp_b", H), ("a_bq", H),
        ("a_const", H), ("r_bq", H), ("r_const", H), ("mg_b", H),
        ("f_b1", 4 * H), ("f_b2", H),
        ("sln_g", H), ("sln_b", H), ("n1_g", H), ("n1_b", H),
        ("n2_g", H), ("n2_b", H), ("n3_g", H), ("n3_b", H),
    ]:
        V[name] = vec_tile(name, rows)
    for i, c in enumerate(COMP):
        V[f"c{i}_b1"] = vec_tile(f"c{i}_b1", c)
        V[f"c{i}_b2"] = vec_tile(f"c{i}_b2", H)

    ones_col = pv.tile([P, 1], FR, tag="ones_col")
    nc.sync.dma_start(out=ones_col[:], in_=D["ones"][:, :])
    eps_t = pv.tile([1, 1], F32, tag="eps")
    nc.vector.memset(eps_t[:], EPS)
    ident = pv.tile([P, P], F32, tag="ident")
    make_identity(nc, ident[:])

    # ---------- helpers ----------
    def mm(ps, steps, nch=NCH):
        """ps[M,N] = sum_k steps[k].lhsT.T @ steps[k].rhs ; chunks the moving dim."""
        n = ps.shape[-1]
        K = len(steps)
        for c0 in range(0, n, nch):
            ce = min(c0 + nch, n)
            for k, (lt, rt) in enumerate(steps):
                nc.tensor.matmul(ps[:, c0:ce], _fr(lt), _fr(rt[:, c0:ce]),
                                 start=(k == 0), stop=(k == K - 1))

    def load_wblocks(pool, dram_ap, nk, cols, tag, c0=0, bufs=1):
        """Load nk row-blocks [P, cols] of a pre-transposed weight, cols [c0, c0+cols)."""
        tiles = []
        for k in range(nk):
            t = pool.tile([P, cols], FR, tag=f"{tag}{k}", bufs=bufs,
                          name=f"{tag}{k}_{c0}")
            nc.sync.dma_start(out=t[:], in_=dram_ap[k * P:(k + 1) * P, c0:c0 + cols])
            tiles.append(t)
        return tiles

    def proj(wname, rhs_tiles, epilogue, pool, ppool, tag, nk=HT, mh=4, wbufs=2):
        """out[m] = epilogue(m, psum(W^T[:,m] @ rhs)), streaming W in col-halves.

        mh: m-tiles per column group (4 -> [P,512] blocks).
        """
        for half in range(HT // mh):
            wb = load_wblocks(pool, D[wname], nk, mh * P, tag, c0=half * mh * P,
                              bufs=wbufs)
            for ml in range(mh):
                m = half * mh + ml
                ps = ppool.tile([P, T], F32, tag="pbig", name=f"{tag}ps{m}")
                mm(ps, [(wb[k][:, ml * P:(ml + 1) * P], rhs_tiles[k][:])
                        for k in range(nk)])
                epilogue(m, ps)

    def layer_norm(z, gname, pools, mk_out, Tn=T, round_out=True):
        """Feature-dim (partition) LN. z: list of HT [P,Tn] tiles.
        mk_out(k) -> output tile. pools = (pp_stat, lnsq, pbc)."""
        pp_stat, lnsq, pbc = pools
        nchunk = max(1, Tn // NCH)
        cw = min(Tn, NCH)
        ps_s = [pp_stat.tile([1, cw], F32, tag="st", name=f"lnps_s{c}") for c in range(nchunk)]
        ps_q = [pp_stat.tile([1, cw], F32, tag="st", name=f"lnps_q{c}") for c in range(nchunk)]
        for c in range(nchunk):
            for k in range(HT):
                nc.tensor.matmul(ps_s[c][:, :], _fr(ones_col[:, 0:1]),
                                 _fr(z[k][:, c * cw:(c + 1) * cw]),
                                 start=(k == 0), stop=(k == HT - 1))
        for k in range(HT):
            sq = lnsq.tile([P, Tn], F32, tag="lnsq")
            nc.vector.tensor_mul(_fr(sq[:]), z[k][:], z[k][:])
            for c in range(nchunk):
                nc.tensor.matmul(ps_q[c][:, :], _fr(ones_col[:, 0:1]),
                                 _fr(sq[:, c * cw:(c + 1) * cw]),
                                 start=(k == 0), stop=(k == HT - 1))
        rstd = lnsq.tile([1, Tn], F32, tag="rstd", bufs=1, name="rstd")
        mr = lnsq.tile([1, Tn], F32, tag="mr", bufs=1, name="mr")
        for c in range(nchunk):
            cs = slice(c * cw, (c + 1) * cw)
            mean_c = lnsq.tile([1, cw], F32, tag="mean", bufs=2, name="mean_c")
            var_c = lnsq.tile([1, cw], F32, tag="var", bufs=2, name="var_c")
            nc.scalar.activation(mean_c[:], ps_s[c][:], AF.Copy, bias=0.0, scale=1.0 / H)
            nc.vector.tensor_mul(var_c[:], mean_c[:], mean_c[:])
            nc.vector.scalar_tensor_tensor(out=var_c[:], in0=ps_q[c][:], scalar=1.0 / H,
                                           in1=var_c[:], op0=OP.mult, op1=OP.subtract)
            nc.scalar.activation(var_c[:], var_c[:], AF.Sqrt, bias=eps_t[:, 0:1])
            nc.vector.reciprocal(rstd[:, cs], var_c[:])
            nc.vector.tensor_mul(mr[:, cs], mean_c[:], rstd[:, cs])
        bc_r = pbc.tile([P, Tn], F32, tag="bc", name="bc_r")
        bc_mr = pbc.tile([P, Tn], F32, tag="bc", name="bc_mr")
        bcast(bc_r[:], rstd[0:1, 0:Tn], P, Tn, "r")
        bcast(bc_mr[:], mr[0:1, 0:Tn], P, Tn, "mr")
        g_t, b_t = V[f"{gname}_g"], V[f"{gname}_b"]
        outs = []
        cast = _fr if round_out else (lambda a: a)
        for k in range(HT):
            o = mk_out(k)
            nc.vector.tensor_mul(cast(o[:]), z[k][:], bc_r[:])
            nc.vector.tensor_sub(cast(o[:]), o[:], bc_mr[:])
            if not fl[f"{gname}_trivial"]:
                nc.vector.tensor_scalar(out=cast(o[:]), in0=o[:],
                                        scalar1=g_t[:, k:k + 1],
                                        scalar2=b_t[:, k:k + 1], op0=OP.mult, op1=OP.add)
            outs.append(o)
        return outs

    # =========================================================================
    # x^T  (resid generation 1)
    # =========================================================================
    # =========================================================================
    # Stage A: SSM layer
    # =========================================================================
    with tc.tile_pool(name="ssm2", bufs=1) as ssm2, \
         tc.tile_pool(name="ppA", bufs=2, space="PSUM") as ppA:
        states = ssm2.tile([P, T], F32, tag="states")
        wC = ssm2.tile([S, H], FR, tag="wC")
        nc.sync.dma_start(out=wC[:], in_=D["wCT"][:, :])
        with tc.tile_pool(name="ssm1", bufs=1) as ssm1:
            wsg = load_wblocks(ssm1, D["wsgT"], HT, S, "wsg")
            wB = load_wblocks(ssm1, D["wBT"], HT, S, "wB")

            psG = ppA.tile([P, T], F32, tag="pbig")
            mm(psG, [(wsg[k][:], xs[k][:]) for k in range(HT)])
            gate = ssm1.tile([P, T], F32, tag="gate")
            nc.scalar.activation(gate[:], psG[:], AF.Sigmoid, bias=V["sg_b"][:, 0:1])

            psB = ppA.tile([P, T], F32, tag="pbig")
            mm(psB, [(wB[k][:], xs[k][:]) for k in range(HT)])
            u = ssm1.tile([P, T], F32, tag="u")
            nc.vector.tensor_mul(u[:], gate[:], psB[:])

            nc.vector.tensor_tensor_scan(_fr(states[:]),
                                         V["A"][:, 0:1].to_broadcast((P, T)), u[:],
                                         0.0, op0=OP.mult, op1=OP.add)

        with tc.tile_pool(name="lnzA", bufs=8) as lnz, \
             tc.tile_pool(name="lnsqA", bufs=3) as lnsq, \
             tc.tile_pool(name="bcA", bufs=2) as pbc, \
             tc.tile_pool(name="lnoA", bufs=8) as lnout, \
             tc.tile_pool(name="ppstA", bufs=4, space="PSUM") as ppst:
            z1 = []
            for m in range(HT):
                psY = ppA.tile([P, T], F32, tag="pbig", name=f"psY{m}")
                mm(psY, [(wC[:, m * P:(m + 1) * P], states[:])])
                zm = lnz.tile([P, T], F32, tag="z", name=f"z1_{m}")
                nc.vector.scalar_tensor_tensor(out=_fr(zm[:]), in0=xs[m][:],
                                               scalar=V["Dp1"][:, m:m + 1], in1=psY[:],
                                               op0=OP.mult, op1=OP.add)
                z1.append(zm)
            ln1 = layer_norm(z1, "sln", (ppst, lnsq, pbc),
                             lambda k: lnout.tile([P, T], F32, tag="ln1", name=f"ln1_{k}"))

            with tc.tile_pool(name="wouA", bufs=1) as wpo:
                z2 = []

                def ep_outp(m, ps):
                    zm = lnz.tile([P, T], F32, tag="z", name=f"z2_{m}")
                    if fl["outp_b_zero"]:
                        nc.vector.tensor_add(_fr(zm[:]), ps[:], xs[m][:])
                    else:
                        nc.vector.scalar_tensor_tensor(out=_fr(zm[:]), in0=ps[:],
                                                       scalar=V["outp_b"][:, m:m + 1],
                                                       in1=xs[m][:], op0=OP.add, op1=OP.add)
                    z2.append(zm)

                proj("outp_wT", ln1, ep_outp, wpo, ppA, "wou", wbufs=1)
            # x1 = resid generation 2
            x1 = layer_norm(z2, "n1", (ppst, lnsq, pbc),
                            lambda k: rtile(k, f"x1_{k}"))

    # =========================================================================
    # Stage B: self-attention
    # =========================================================================
    Oh = [rtile(g, f"oh{g}") for g in range(HT)]  # resid generation 3
    with tc.tile_pool(name="pQ", bufs=1) as pQ, \
         tc.tile_pool(name="pK", bufs=1) as pK, \
         tc.tile_pool(name="pV", bufs=1) as pV:
        with tc.tile_pool(name="wqkv", bufs=1) as wqkv, \
             tc.tile_pool(name="ppB1", bufs=2, space="PSUM") as ppB1:
            Qh, Kh, Vp = [], [], []

            def ep_q(m, ps):
                qm = pQ.tile([P, T], F32, tag=f"q{m}", name=f"q{m}")
                if fl["a_bq_zero"]:
                    nc.vector.tensor_copy(out=_fr(qm[:]), in_=ps[:])
                else:
                    nc.vector.tensor_scalar_add(_fr(qm[:]), ps[:], V["a_bq"][:, m:m + 1])
                Qh.append(qm)

            def ep_k(m, ps):
                km = pK.tile([P, T], F32, tag=f"k{m}", name=f"k{m}")
                nc.vector.tensor_copy(out=_fr(km[:]), in_=ps[:])
                Kh.append(km)

            proj("a_wqT", x1, ep_q, wqkv, ppB1, "wq", wbufs=1)
            proj("a_wkT", x1, ep_k, wqkv, ppB1, "wq", wbufs=1)
            # V token-major, with a ones column appended per head
            for kt in range(HT):
                vt = pV.tile([P, NH * (DH + 1)], FR, tag=f"v{kt}", name=f"v{kt}")
                nc.sync.dma_start(
                    out=vt[:].rearrange("p (h c) -> p h c", c=DH + 1)[:, :, DH:DH + 1],
                    in_=D["ones16"].rearrange("p (h o) -> p h o", o=1))
                Vp.append(vt)
            for vh in range(2):
                wvb = load_wblocks(wqkv, D["a_wvT"], HT, 512, "wq", c0=vh * 512)
                hs = 512 // (DH + 1) + 1  # 8 heads per half
                for kt in range(HT):
                    ps = ppB1.tile([P, 512], F32, tag="pvh", name=f"psV{vh}_{kt}")
                    mm(ps, [(x1[k][:, kt * P:(kt + 1) * P], wvb[k][:])
                            for k in range(HT)])
                    nc.vector.tensor_copy(
                        out=Vp[kt][:].rearrange("p (h c) -> p h c", c=DH + 1)[
                            :, 8 * vh:8 * (vh + 1), 0:DH],
                        in_=ps[:].rearrange("p (h c) -> p h c", c=DH)[:, :, :])

        with tc.tile_pool(name="pexp", bufs=4) as pexp, \
             tc.tile_pool(name="pbcB", bufs=2) as pbcB, \
             tc.tile_pool(name="ppSC", bufs=2, space="PSUM") as ppSC, \
             tc.tile_pool(name="ppAV", bufs=2, space="PSUM") as ppAV:
            for h in range(NH):
                g, ho = h // 2, (h % 2) * DH
                psA = ppAV.tile([DH + 1, T], F32, tag="pav", name=f"pav{h}")
                for kt in range(HT):
                    psS = ppSC.tile([P, T], F32, tag="psc", name=f"psc{h}_{kt}")
                    mm(psS, [(Kh[g][ho:ho + DH, kt * P:(kt + 1) * P],
                              Qh[g][ho:ho + DH, :])])
                    et = pexp.tile([P, T], FR, tag="exp", name=f"exp{h}_{kt}")
                    nc.scalar.activation(et[:], psS[:], AF.Exp, scale=1.0 / 8.0)
                    vslice = Vp[kt][:, h * (DH + 1):(h + 1) * (DH + 1)]
                    for c0 in range(0, T, NCH):
                        nc.tensor.matmul(psA[:, c0:c0 + NCH], _fr(vslice),
                                         _fr(et[:, c0:c0 + NCH]),
                                         start=(kt == 0), stop=(kt == HT - 1))
                rcp = sm.tile([1, T], F32, tag="stat", name=f"rcp{h}")
                nc.vector.reciprocal(rcp[:], psA[DH:DH + 1, :])
                rb = pbcB.tile([DH, T], F32, tag="rb", name=f"rb{h}")
                bcast(rb[:], rcp[0:1, :], DH, T, "rc")
                nc.vector.tensor_mul(_fr(Oh[g][ho:ho + DH, :]), psA[0:DH, :], rb[:])

    with tc.tile_pool(name="lnzB", bufs=8) as lnz, \
         tc.tile_pool(name="lnsqB", bufs=3) as lnsq, \
         tc.tile_pool(name="bcB2", bufs=2) as pbc, \
         tc.tile_pool(name="woB", bufs=1) as wpo, \
         tc.tile_pool(name="ppB3", bufs=2, space="PSUM") as ppB3, \
         tc.tile_pool(name="ppstB", bufs=4, space="PSUM") as ppst:
        z3 = []

        def ep_wo(m, ps):
            zm = lnz.tile([P, T], F32, tag="z", name=f"z3_{m}")
            if fl["a_const_zero"]:
                nc.vector.tensor_add(_fr(zm[:]), ps[:], x1[m][:])
            else:
                nc.vector.scalar_tensor_tensor(out=_fr(zm[:]), in0=ps[:],
                                               scalar=V["a_const"][:, m:m + 1],
                                               in1=x1[m][:], op0=OP.add, op1=OP.add)
            z3.append(zm)

        proj("a_woT", Oh, ep_wo, wpo, ppB3, "wo")
        # x2 = resid generation 4
        x2 = layer_norm(z3, "n2", (ppst, lnsq, pbc),
                        lambda k: rtile(k, f"x2_{k}"))

    # =========================================================================
    # Stage C: hierarchical memory retrieval + gated merge
    # =========================================================================
    cstk = ExitStack()
    with cstk:
        pKr = cstk.enter_context(tc.tile_pool(name="pKr", bufs=1))
        pVr = cstk.enter_context(tc.tile_pool(name="pVr", bufs=1))
        Kr, Vr = {}, {}
        with tc.tile_pool(name="pc", bufs=1) as pc:
            chat = {}
            with tc.tile_pool(name="cw", bufs=1) as cw, \
                 tc.tile_pool(name="cmid", bufs=1) as cmid, \
                 tc.tile_pool(name="ppC1", bufs=6, space="PSUM") as ppC1:
                for i, cwid in enumerate(COMP):
                    ct = cwid // P
                    with tc.tile_pool(name=f"pmT{i}", bufs=1) as pmT:
                        mT = load_wblocks(pmT, D[f"m{i}T"], HT, 256, "mT")
                        w1 = load_wblocks(cw, D[f"c{i}_w1T"], HT, cwid, "cwx")
                        mid = []
                        for cm in range(ct):
                            ps = ppC1.tile([P, 256], F32, tag="pc1", name=f"pm{i}_{cm}")
                            mm(ps, [(w1[k][:, cm * P:(cm + 1) * P], mT[k][:])
                                    for k in range(HT)])
                            md = cmid.tile([P, 256], FR, tag=f"mid{cm}", name=f"mid{i}_{cm}")
                            nc.scalar.activation(md[:], ps[:], AF.Relu,
                                                 bias=V[f"c{i}_b1"][:, cm:cm + 1])
                            mid.append(md)
                        w2 = load_wblocks(cw, D[f"c{i}_w2T"], ct, H, "cwx")
                        for m in range(HT):
                            ps = ppC1.tile([P, 256], F32, tag="pc1", name=f"pc{i}_{m}")
                            mm(ps, [(w2[k][:, m * P:(m + 1) * P], mid[k][:])
                                    for k in range(ct)])
                            cm_t = pc.tile([P, 256], F32, tag=f"c{i}_{m}", name=f"c{i}_{m}")
                            if fl[f"c{i}_b2_zero"]:
                                nc.vector.tensor_copy(out=_fr(cm_t[:]), in_=ps[:])
                            else:
                                nc.vector.tensor_scalar_add(_fr(cm_t[:]), ps[:],
                                                            V[f"c{i}_b2"][:, m:m + 1])
                            chat.setdefault(i, []).append(cm_t)
            with tc.tile_pool(name="rkv", bufs=1) as rkv, \
                 tc.tile_pool(name="ppC2", bufs=2, space="PSUM") as ppC2:
                wkr = load_wblocks(rkv, D["r_wkT"], HT, H, "rkv")
                for i in range(3):
                    Kr[i] = []
                    for m in range(HT):
                        ps = ppC2.tile([P, 256], F32, tag="pkv", name=f"pk{i}_{m}")
                        mm(ps, [(wkr[k][:, m * P:(m + 1) * P], chat[i][k][:])
                                for k in range(HT)])
                        kt_ = pKr.tile([P, 256], F32, tag=f"kr{i}_{m}", name=f"kr{i}_{m}")
                        nc.vector.tensor_copy(out=_fr(kt_[:]), in_=ps[:])
                        Kr[i].append(kt_)
                wvr = load_wblocks(rkv, D["r_wvT"], HT, H, "rkv")
                for i in range(3):
                    Vr[i] = []
                    for kvt in range(2):
                        ps = ppC2.tile([P, T], F32, tag="pkv2", name=f"pv{i}_{kvt}")
                        mm(ps, [(chat[i][k][:, kvt * P:(kvt + 1) * P], wvr[k][:])
                                for k in range(HT)])
                        vt = pVr.tile([P, T], F32, tag=f"vr{i}_{kvt}", name=f"vr{i}_{kvt}")
                        nc.vector.tensor_copy(out=_fr(vt[:]), in_=ps[:])
                        Vr[i].append(vt)
        pQr = cstk.enter_context(tc.tile_pool(name="pQr", bufs=1))
        with tc.tile_pool(name="rwq", bufs=1) as rwq, \
             tc.tile_pool(name="ppC3", bufs=3, space="PSUM") as ppC3:
            Qr = []

            def ep_qr(m, ps):
                qm = pQr.tile([P, T], F32, tag=f"qr{m}", name=f"qr{m}")
                if fl["r_bq_zero"]:
                    nc.vector.tensor_copy(out=_fr(qm[:]), in_=ps[:])
                else:
                    nc.vector.tensor_scalar_add(_fr(qm[:]), ps[:], V["r_bq"][:, m:m + 1])
                Qr.append(qm)

            proj("r_wqT", x2, ep_qr, rwq, ppC3, "rwq")

        Or = [rtile(m, f"orr{m}") for m in range(HT)]  # resid generation 5
        with tc.tile_pool(name="pexpR", bufs=4) as pexpR, \
             tc.tile_pool(name="ptwR", bufs=2) as ptw, \
             tc.tile_pool(name="pbcR", bufs=3) as pbcR, \
             tc.tile_pool(name="ppSCr", bufs=1, space="PSUM") as ppSCr, \
             tc.tile_pool(name="ppAVr", bufs=2, space="PSUM") as ppAVr, \
             tc.tile_pool(name="ppsum", bufs=2, space="PSUM") as ppsum:
            for i in range(3):
                for r in range(RH):
                    ets = []
                    for kvt in range(2):
                        psS = ppSCr.tile([P, T], F32, tag="psc", name=f"rsc{i}{r}{kvt}")
                        mm(psS, [(Kr[i][2 * r + kc][:, kvt * P:(kvt + 1) * P],
                                  Qr[2 * r + kc][:]) for kc in range(2)])
                        et = pexpR.tile([P, T], FR, tag="expr", name=f"re{i}{r}{kvt}")
                        nc.scalar.activation(et[:], psS[:], AF.Exp, scale=1.0 / 16.0)
                        ets.append(et)
                    rcp = sm.tile([1, T], F32, tag="stat", name=f"rcpr{i}{r}")
                    for c0 in range(0, T, NCH):
                        psZ = ppsum.tile([1, NCH], F32, tag="pz", name=f"rz{i}{r}{c0}")
                        for kvt in range(2):
                            nc.tensor.matmul(psZ[:, :], _fr(ones_col[:, 0:1]),
                                             _fr(ets[kvt][:, c0:c0 + NCH]),
                                             start=(kvt == 0), stop=(kvt == 1))
                        nc.vector.reciprocal(rcp[:, c0:c0 + NCH], psZ[:, :])
                    rb = pbcR.tile([P, T], F32, tag="rbr", name=f"rbr{i}{r}")
                    bcast(rb[:], rcp[0:1, :], P, T, "rr")
                    for md in range(2):
                        psA = ppAVr.tile([P, T], F32, tag="pav", name=f"rav{i}{r}{md}")
                        col = RDH * r + P * md
                        for c0 in range(0, T, NCH):
                            for kvt in range(2):
                                nc.tensor.matmul(psA[:, c0:c0 + NCH],
                                                 _fr(Vr[i][kvt][:, col:col + P]),
                                                 _fr(ets[kvt][:, c0:c0 + NCH]),
                                                 start=(kvt == 0), stop=(kvt == 1))
                        dst = Or[2 * r + md]
                        if i == 0:
                            nc.vector.tensor_mul(_fr(dst[:]), psA[:], rb[:])
                        else:
                            tw = ptw.tile([P, T], F32, tag="tw", name=f"tw{i}{r}{md}")
                            nc.vector.tensor_mul(tw[:], psA[:], rb[:])
                            nc.vector.tensor_add(_fr(dst[:]), dst[:], tw[:])

    with tc.tile_pool(name="pcomb", bufs=1) as pcomb:
        with tc.tile_pool(name="rwo", bufs=1) as rwo, \
             tc.tile_pool(name="ppC5", bufs=3, space="PSUM") as ppC5:
            comb = []

            def ep_ro(m, ps):
                cm_ = pcomb.tile([P, T], F32, tag=f"cb{m}", name=f"cb{m}")
                if fl["r_const_zero"]:
                    nc.scalar.activation(_fr(cm_[:]), ps[:], AF.Copy, bias=0.0, scale=1.0 / 3.0)
                else:
                    nc.vector.tensor_scalar(out=_fr(cm_[:]), in0=ps[:], scalar1=1.0 / 3.0,
                                            scalar2=V["r_const"][:, m:m + 1],
                                            op0=OP.mult, op1=OP.add)
                comb.append(cm_)

            proj("r_woT", Or, ep_ro, rwo, ppC5, "rwo")

        x3 = []
        with tc.tile_pool(name="mgw", bufs=1) as mgw, \
             tc.tile_pool(name="pgw", bufs=2) as pgw, \
             tc.tile_pool(name="ptmp", bufs=2) as ptmp, \
             tc.tile_pool(name="ppC6", bufs=3, space="PSUM") as ppC6:
            for half in range(2):
                wb = load_wblocks(mgw, D["mg_wT"], 2 * HT, 4 * P, "mg", c0=half * 4 * P,
                                  bufs=2)
                for ml in range(4):
                    m = half * 4 + ml
                    ps = ppC6.tile([P, T], F32, tag="pbig", name=f"mgps{m}")
                    steps = [(wb[k][:, ml * P:(ml + 1) * P], x2[k][:]) for k in range(HT)]
                    steps += [(wb[HT + k][:, ml * P:(ml + 1) * P], comb[k][:])
                              for k in range(HT)]
                    mm(ps, steps)
                    gw = pgw.tile([P, T], F32, tag="gw", name=f"gw{m}")
                    nc.scalar.activation(gw[:], ps[:], AF.Sigmoid, bias=V["mg_b"][:, m:m + 1])
                    d = ptmp.tile([P, T], F32, tag="d", name=f"d{m}")
                    nc.vector.tensor_sub(d[:], x2[m][:], comb[m][:])
                    nc.vector.tensor_mul(d[:], gw[:], d[:])
                    s = ptmp.tile([P, T], F32, tag="s", name=f"s{m}")
                    nc.vector.tensor_add(s[:], x2[m][:], comb[m][:])
                    xm = rtile(m, f"x3_{m}")  # resid generation 6
                    nc.vector.tensor_add(_fr(xm[:]), s[:], d[:])
                    x3.append(xm)

    # =========================================================================
    # Stage D: FFN in token-halves + final LN + transpose to [T, H]
    # =========================================================================
    TH = T // 2
    with tc.tile_pool(name="ph", bufs=1) as ph:
        hts = {0: [], 1: []}
        with tc.tile_pool(name="fw1", bufs=2) as fw1, \
             tc.tile_pool(name="ppD1", bufs=4, space="PSUM") as ppD1:
            for mg_i in range(8):
                wblk = load_wblocks(fw1, D["f_w1T"], HT, 512, "w1s",
                                    c0=mg_i * 512, bufs=2)
                for th in range(2):
                    c0 = th * TH
                    for ml in range(4):
                        m_abs = mg_i * 4 + ml
                        ps = ppD1.tile([P, TH], F32, tag="p1", name=f"f1ps{th}_{m_abs}")
                        mm(ps, [(wblk[k][:, ml * P:(ml + 1) * P],
                                 x3[k][:, c0:c0 + TH]) for k in range(HT)])
                        htile = ph.tile([P, TH], BF16, tag=f"h{th}_{m_abs}",
                                        name=f"h{th}_{m_abs}")
                        nc.scalar.activation(htile[:], ps[:], AF.Gelu,
                                             bias=V["f_b1"][:, m_abs:m_abs + 1])
                        hts[th].append(htile)
        for th in range(2):
            c0 = th * TH
            with tc.tile_pool(name="lnzD", bufs=8) as lnz:
                z4 = []
                with tc.tile_pool(name="fw2", bufs=4) as fw2, \
                     tc.tile_pool(name="ppD2", bufs=1, space="PSUM") as ppD2:
                    pso = [ppD2.tile([P, TH], F32, tag=f"p2_{m}", name=f"pso{th}_{m}")
                           for m in range(HT)]
                    for k2 in range(4 * HT):
                        wt = fw2.tile([P, H], BF16, tag="w2s", name=f"w2s{th}_{k2}")
                        nc.sync.dma_start(out=wt[:], in_=D["f_w2T"][k2 * P:(k2 + 1) * P, :])
                        for mo in range(HT):
                            nc.tensor.matmul(pso[mo][:, :],
                                             wt[:, mo * P:(mo + 1) * P],
                                             hts[th][k2][:],
                                             start=(k2 == 0), stop=(k2 == 4 * HT - 1))
                    for mo in range(HT):
                        zm = lnz.tile([P, TH], F32, tag="z", name=f"z4_{th}_{mo}")
                        if fl["f_b2_zero"]:
                            nc.vector.tensor_add(_fr(zm[:]), pso[mo][:], x3[mo][:, c0:c0 + TH])
                        else:
                            nc.vector.scalar_tensor_tensor(out=_fr(zm[:]), in0=pso[mo][:],
                                                           scalar=V["f_b2"][:, mo:mo + 1],
                                                           in1=x3[mo][:, c0:c0 + TH],
                                                           op0=OP.add, op1=OP.add)
                        z4.append(zm)
                with tc.tile_pool(name="lnsqD", bufs=2) as lnsq, \
                     tc.tile_pool(name="bcD", bufs=2) as pbc, \
                     tc.tile_pool(name="lnoD", bufs=8) as lnout, \
                     tc.tile_pool(name="stg", bufs=2) as stg_pool, \
                     tc.tile_pool(name="ppstD", bufs=4, space="PSUM") as ppst, \
                     tc.tile_pool(name="ppT", bufs=4, space="PSUM") as ppT:
                    fin = layer_norm(z4, "n3", (ppst, lnsq, pbc),
                                     lambda k: lnout.tile([P, TH], F32, tag="fin",
                                                          name=f"fin{th}_{k}"),
                                     Tn=TH, round_out=False)
                    for tt in range(TH // P):
                        stg = stg_pool.tile([P, H], F32, tag="stg", name=f"stg{th}_{tt}")
                        for k2 in range(HT):
                            psT = ppT.tile([P, P], F32, tag="pt", name=f"pT{th}_{tt}_{k2}")
                            nc.tensor.transpose(psT[:, :],
                                                fin[k2][:, tt * P:(tt + 1) * P],
                                                ident[:])
                            nc.vector.tensor_copy(out=stg[:, k2 * P:(k2 + 1) * P],
                                                  in_=psT[:, :])
                        row0 = c0 + tt * P
                        nc.sync.dma_start(out=out_d[row0:row0 + P, :], in_=stg[:])
    ctx.close()


# =============================================================================
# Host side
# =============================================================================
_CACHE = {}


def _flags(g):
    def zero(a):
        return bool(np.all(a == 0.0))

    fl = {}
    for n in ("sln", "n1", "n2", "n3"):
        fl[f"{n}_trivial"] = bool(np.all(g[f"{n}_g"] == 1.0) and zero(g[f"{n}_b"]))
    fl["outp_b_zero"] = zero(g["outp_b"])
    wq_b, wk_b, wv_b = np.split(g["attn_in_b"], 3, 0)
    fl["a_bq_zero"] = zero(wq_b)
    a_const = wv_b @ g["attn_out_w"].T + g["attn_out_b"]
    fl["a_const_zero"] = zero(a_const)
    rq_b, rk_b, rv_b = np.split(g["retr_in_b"], 3, 0)
    fl["r_bq_zero"] = zero(rq_b)
    r_const = rv_b @ g["retr_out_w"].T + g["retr_out_b"]
    fl["r_const_zero"] = zero(r_const)
    for i in range(3):
        fl[f"c{i}_b2_zero"] = zero(g[f"c{i}_b2"])
    fl["f_b2_zero"] = zero(g["ffn_b2"])
    return fl, a_const, r_const


def kernel(**inputs):
    g = {k: np.ascontiguousarray(np.asarray(v, dtype=np.float32)) for k, v in inputs.items()}
    fl, a_const, r_const = _flags(g)

    key = tuple(sorted(fl.items()))
    if key not in _CACHE:
        _CACHE[key] = build_nc(fl)
    nc = _CACHE[key]

    def tr(a):
        return np.ascontiguousarray(a.T)

    wq, wk, wv = np.split(g["attn_in_w"], 3, 0)
    rq, rk, rv = np.split(g["retr_in_w"], 3, 0)
    shared = {
        "wsgT": tr(g["sgate_w"]), "wBT": tr(g["B_w"]), "wCT": tr(g["C_w"]),
        "A": np.exp(g["A_log"]).reshape(S, 1), "sg_b": g["sgate_b"].reshape(S, 1),
        "Dp1": (g["D"] + 1.0).reshape(H, 1),
        "outp_wT": tr(g["outp_w"]), "outp_b": g["outp_b"].reshape(H, 1),
        "a_wqT": tr(wq), "a_wkT": tr(wk), "a_wvT": tr(wv),
        "a_bq": np.split(g["attn_in_b"], 3, 0)[0].reshape(H, 1),
        "a_woT": tr(g["attn_out_w"]), "a_const": a_const.reshape(H, 1),
        "r_wqT": tr(rq), "r_wkT": tr(rk), "r_wvT": tr(rv),
        "r_bq": np.split(g["retr_in_b"], 3, 0)[0].reshape(H, 1),
        "r_woT": tr(g["retr_out_w"]), "r_const": r_const.reshape(H, 1),
        "mg_wT": tr(g["mg_w"]), "mg_b": g["mg_b"].reshape(H, 1),
        "f_w1T": tr(g["ffn_w1"]), "f_b1": g["ffn_b1"].reshape(4 * H, 1),
        "f_b2": g["ffn_b2"].reshape(H, 1),
    }
    for i in range(3):
        shared[f"c{i}_w1T"] = tr(g[f"c{i}_w1"])
        shared[f"c{i}_b1"] = g[f"c{i}_b1"].reshape(COMP[i], 1)
        shared[f"c{i}_w2T"] = tr(g[f"c{i}_w2"])
        shared[f"c{i}_b2"] = g[f"c{i}_b2"].reshape(H, 1)
    for n in ("sln", "n1", "n2", "n3"):
        shared[f"{n}_g"] = g[f"{n}_g"].reshape(H, 1)
        shared[f"{n}_b"] = g[f"{n}_b"].reshape(H, 1)
    shared["ones"] = np.ones((P, 1), np.float32)
    shared["ones16"] = np.ones((P, NH), np.float32)
    shared = {k: np.ascontiguousarray(v.astype(np.float32)) for k, v in shared.items()}
    import ml_dtypes
    shared["f_w2T"] = np.ascontiguousarray(tr(g["ffn_w2"]).astype(ml_dtypes.bfloat16))

    in_maps = []
    for b in range(B):
        m = dict(shared)
        m["xT"] = tr(g["x"][b])
        for i in range(3):
            m[f"m{i}T"] = tr(g[f"mem{i}"][b, -256:, :])
        in_maps.append(m)

    trace = os.environ.get("KERNEL_TRACE", "0") == "1"
    res = bass_utils.run_bass_kernel_spmd(nc, in_maps, core_ids=list(range(B)),
                                          trace=trace)
    global LAST_RESULTS
    LAST_RESULTS = res
    out = np.stack([res.results[b]["out"] for b in range(B)], axis=0)
    return out


LAST_RESULTS = None


def bench(n_iter=6, **inputs):
    """Time the on-device execution with device-resident inputs (excludes
    host->device transfer and the axon tunnel's fixed per-dispatch cost;
    see the scan-diff measurement below). Returns (best_seconds, out)."""
    import time

    import jax
    import jax.numpy as jnp
    from jax.sharding import Mesh, PartitionSpec
    from jax.experimental.shard_map import shard_map
    from concourse import bass2jax

    g = {k: np.ascontiguousarray(np.asarray(v, dtype=np.float32)) for k, v in inputs.items()}
    fl, a_const, r_const = _flags(g)
    key = tuple(sorted(fl.items()))
    if key not in _CACHE:
        _CACHE[key] = build_nc(fl)
    nc = _CACHE[key]
    in_maps = _in_maps(g, a_const, r_const)

    bass2jax.install_neuronx_cc_hook()
    import concourse.mybir as mybir_
    in_names, out_names, out_avals, zero_outs = [], [], [], []
    for alloc in nc.m.functions[0].allocations:
        if not isinstance(alloc, mybir_.MemoryLocationSet):
            continue
        name = alloc.memorylocations[0].name
        pid_name = nc.partition_id_tensor.name if nc.partition_id_tensor else None
        if alloc.kind == "ExternalInput":
            if name != pid_name:
                in_names.append(name)
        elif alloc.kind == "ExternalOutput":
            out_names.append(name)
            np_dt = mybir_.dt.np(alloc.dtype)
            out_avals.append(jax.core.ShapedArray(tuple(alloc.tensor_shape), np_dt))
            zero_outs.append(np.zeros(tuple(alloc.tensor_shape), np_dt))
    n_params = len(in_names)
    all_names = in_names + out_names
    if nc.partition_id_tensor is not None:
        all_names = all_names + [nc.partition_id_tensor.name]

    def _body(*args):
        operands = list(args)
        if nc.partition_id_tensor is not None:
            operands.append(bass2jax.partition_id_tensor())
        outs = bass2jax._bass_exec_p.bind(
            *operands, out_avals=tuple(out_avals), in_names=tuple(all_names),
            out_names=tuple(out_names), lowering_input_output_aliases=(),
            sim_require_finite=True, sim_require_nnan=True, nc=nc)
        return tuple(outs)

    devices = jax.devices()[:B]
    mesh = Mesh(np.asarray(devices), ("core",))
    nin = n_params + len(out_names)
    sh = jax.sharding.NamedSharding(mesh, PartitionSpec("core"))
    concat_in = [np.concatenate([np.asarray(in_maps[c][i_name])
                                 for c in range(B)], axis=0) for i_name in in_names]
    concat_zeros = [np.zeros((B * z.shape[0], *z.shape[1:]), z.dtype) for z in zero_outs]
    dev_in = [jax.device_put(a, sh) for a in concat_in + concat_zeros]
    jax.block_until_ready(dev_in)

    def _compile():
        return jax.jit(shard_map(
            _body, mesh=mesh,
            in_specs=(PartitionSpec("core"),) * nin,
            out_specs=(PartitionSpec("core"),) * len(out_names),
            check_rep=False)).lower(*dev_in).compile()

    try:
        sharded = bass2jax.fast_dispatch_compile(_compile)
    except Exception as e:
        print(f"  (fast dispatch unavailable: {e})")
        sharded = jax.jit(shard_map(_body, mesh=mesh,
                                    in_specs=(PartitionSpec("core"),) * nin,
                                    out_specs=(PartitionSpec("core"),) * len(out_names),
                                    check_rep=False))
    out = None
    for it in range(2):
        t0 = time.perf_counter()
        out = sharded(*dev_in)
        jax.block_until_ready(out)
        dt = time.perf_counter() - t0
        print(f"  iter {it}: {dt * 1e3:.2f} ms (sync round-trip)")
    # amortized per queued call, for reference
    nq = 16
    t0 = time.perf_counter()
    outs = [sharded(*dev_in) for _ in range(nq)]
    jax.block_until_ready(outs)
    print(f"  amortized over {nq} queued calls: "
          f"{(time.perf_counter() - t0) / nq * 1e3:.2f} ms (incl dispatch floor)")

    # On-device execution time, isolated from the axon tunnel's per-dispatch
    # floor: run the whole kernel K times inside ONE NEFF via a hardware loop
    # (tc.For_i around the body; each iteration re-reads the inputs from DRAM
    # and rewrites the output, i.e. K real, serialized full executions), at
    # two different K. The wall-time difference divided by the
    # iteration-count difference cancels the fixed dispatch cost exactly.
    KS, KL = 2, 18

    def _make_loop(klen):
        lkey = (key, klen)
        if lkey not in _CACHE:
            _CACHE[lkey] = build_nc(fl, loop_n=klen)
        lnc = _CACHE[lkey]
        lout_avals = out_avals

        def _lbody(*args):
            operands = list(args)
            if lnc.partition_id_tensor is not None:
                operands.append(bass2jax.partition_id_tensor())
            lall = in_names + out_names
            if lnc.partition_id_tensor is not None:
                lall = lall + [lnc.partition_id_tensor.name]
            outs = bass2jax._bass_exec_p.bind(
                *operands, out_avals=tuple(lout_avals), in_names=tuple(lall),
                out_names=tuple(out_names), lowering_input_output_aliases=(),
                sim_require_finite=True, sim_require_nnan=True, nc=lnc)
            return tuple(outs)

        def _c():
            return jax.jit(shard_map(
                _lbody, mesh=mesh,
                in_specs=(PartitionSpec("core"),) * nin,
                out_specs=(PartitionSpec("core"),) * len(out_names),
                check_rep=False)).lower(*dev_in).compile()

        try:
            return bass2jax.fast_dispatch_compile(_c)
        except Exception:
            return jax.jit(shard_map(_lbody, mesh=mesh,
                                     in_specs=(PartitionSpec("core"),) * nin,
                                     out_specs=(PartitionSpec("core"),) * len(out_names),
                                     check_rep=False))

    loop_s, loop_l = _make_loop(KS), _make_loop(KL)
    out_s = loop_s(*dev_in)
    jax.block_until_ready(out_s)  # warm compile
    jax.block_until_ready(loop_l(*dev_in))
    best_s = best_l = None
    for rnd in range(5):
        t0 = time.perf_counter()
        jax.block_until_ready(loop_s(*dev_in))
        ws = time.perf_counter() - t0
        t0 = time.perf_counter()
        jax.block_until_ready(loop_l(*dev_in))
        wl = time.perf_counter() - t0
        print(f"  loop x{KS}: {ws * 1e3:.2f} ms | loop x{KL}: {wl * 1e3:.2f} ms "
              f"| per-exec {(wl - ws) / (KL - KS) * 1e3:.3f} ms")
        best_s = ws if best_s is None or ws < best_s else best_s
        best_l = wl if best_l is None or wl < best_l else best_l
    per_exec = (best_l - best_s) / (KL - KS)
    print(f"  device exec per iteration (loop-diff): {per_exec * 1e3:.3f} ms")
    res = np.asarray(out_s[0]).reshape(B, T, H)
    return per_exec, res


def _in_maps(g, a_const, r_const):
    def tr(a):
        return np.ascontiguousarray(a.T)

    wq, wk, wv = np.split(g["attn_in_w"], 3, 0)
    rq, rk, rv = np.split(g["retr_in_w"], 3, 0)
    shared = {
        "wsgT": tr(g["sgate_w"]), "wBT": tr(g["B_w"]), "wCT": tr(g["C_w"]),
        "A": np.exp(g["A_log"]).reshape(S, 1), "sg_b": g["sgate_b"].reshape(S, 1),
        "Dp1": (g["D"] + 1.0).reshape(H, 1),
        "outp_wT": tr(g["outp_w"]), "outp_b": g["outp_b"].reshape(H, 1),
        "a_wqT": tr(wq), "a_wkT": tr(wk), "a_wvT": tr(wv),
        "a_bq": np.split(g["attn_in_b"], 3, 0)[0].reshape(H, 1),
        "a_woT": tr(g["attn_out_w"]), "a_const": a_const.reshape(H, 1),
        "r_wqT": tr(rq), "r_wkT": tr(rk), "r_wvT": tr(rv),
        "r_bq": np.split(g["retr_in_b"], 3, 0)[0].reshape(H, 1),
        "r_woT": tr(g["retr_out_w"]), "r_const": r_const.reshape(H, 1),
        "mg_wT": tr(g["mg_w"]), "mg_b": g["mg_b"].reshape(H, 1),
        "f_w1T": tr(g["ffn_w1"]), "f_b1": g["ffn_b1"].reshape(4 * H, 1),
        "f_b2": g["ffn_b2"].reshape(H, 1),
    }
    for i in range(3):
        shared[f"c{i}_w1T"] = tr(g[f"c{i}_w1"])
        shared[f"c{i}_b1"] = g[f"c{i}_b1"].reshape(COMP[i], 1)
        shared[f"c{i}_w2T"] = tr(g[f"c{i}_w2"])
        shared[f"c{i}_b2"] = g[f"c{i}_b2"].reshape(H, 1)
    for n in ("sln", "n1", "n2", "n3"):
        shared[f"{n}_g"] = g[f"{n}_g"].reshape(H, 1)
        shared[f"{n}_b"] = g[f"{n}_b"].reshape(H, 1)
    shared["ones"] = np.ones((P, 1), np.float32)
    shared["ones16"] = np.ones((P, NH), np.float32)
    shared = {k: np.ascontiguousarray(v.astype(np.float32)) for k, v in shared.items()}
    import ml_dtypes
    shared["f_w2T"] = np.ascontiguousarray(tr(g["ffn_w2"]).astype(ml_dtypes.bfloat16))
    in_maps = []
    for b in range(B):
        m = dict(shared)
        m["xT"] = tr(g["x"][b])
        for i in range(3):
            m[f"m{i}T"] = tr(g[f"mem{i}"][b, -256:, :])
        in_maps.append(m)
    return in_maps

